# revision 17
# baseline (speedup 1.0000x reference)
"""Trainium2 Bass kernel for nn_Attention (dense transformer block:
QKV proj + RoPE + causal GQA attention + o_proj), SPMD over 8 NeuronCores.

Sharding: core c -> (batch b = c//4, head-group g = c%4). Each core computes
4 query heads + its kv head for one batch; per s-chunk the 4 head outputs are
AllGather'd (bf16) within the 4-core batch group and each core computes a
disjoint 512-column slice of the o_proj output for that chunk.

v2 vs v1:
- x passed host-transposed (xT [E,S]) -> no PE transposes / PSUM evacuation.
- chunk-outer pipeline: proj(sc) -> attention(sc, 4 heads) -> AllGather(sc)
  -> o_proj(sc-1), so collectives overlap the next chunk's compute.
- bf16 for QT/KT/V/probs/AllGather/o_proj operands (PE rate unchanged,
  halves collective bytes, 2x DVE mask, FWL weight loads).
- causal trim: diagonal k-tiles only compute q-columns >= 128*t.
- o_proj accumulates all 16 head-blocks in PSUM (no DVE adds / outAcc).
- reciprocal_approx_fast instead of DVE reciprocal (3.3us -> ~0.7us).
"""

import sys
import time

sys.path.insert(0, "/opt/trn_rl_repo")

import numpy as np
import ml_dtypes

import concourse.bass as bass
import concourse.mybir as mybir
import concourse.tile as tile
from concourse import bacc
from concourse.masks import make_identity

F32 = mybir.dt.float32
F32R = mybir.dt.float32r
BF16 = mybir.dt.bfloat16
NPBF16 = ml_dtypes.bfloat16
P = 128
HD = 128            # head dim
NHL = 4             # query heads per core
E = 2048            # hidden
DQ = NHL * HD       # 512, local q-projection width / o-slice width
SCALE = 1.0 / np.sqrt(np.float32(HD))
REPLICA_GROUPS = [[0, 1, 2, 3], [4, 5, 6, 7]]
LAG = 4             # pv(kt-LAG) emitted after scores(kt): hides exp+mask


def r32(ap):
    return ap.bitcast(F32R)


def build_program(S=2048, reps=1, n_cores=8):
    """Build the per-core SPMD Bass program. Returns compiled nc."""
    NQC = S // 512       # 512-wide chunks along sequence
    ET = E // P          # 16 tiles along hidden

    nc = bacc.Bacc("TRN2", target_bir_lowering=False, debug=False,
                   num_devices=n_cores)

    xT_in = nc.declare_dram_parameter("xT", [E, S], F32, isOutput=False)
    wqT_in = nc.declare_dram_parameter("wqT", [E, DQ], F32, isOutput=False)
    wkT_in = nc.declare_dram_parameter("wkT", [E, HD], F32, isOutput=False)
    wvT_in = nc.declare_dram_parameter("wvT", [E, HD], F32, isOutput=False)
    woT_in = nc.declare_dram_parameter("woT", [E, DQ], BF16, isOutput=False)
    cosT_in = nc.declare_dram_parameter("cosT", [HD, S], BF16, isOutput=False)
    sinT_in = nc.declare_dram_parameter("sinT", [HD, S], BF16, isOutput=False)
    out_d = nc.declare_dram_parameter("out", [DQ, S], F32, isOutput=True)

    with tile.TileContext(nc) as tc:
        with nc.allow_low_precision(reason="bf16/f32r attention pipeline"):
            _emit(tc, nc, S, NQC, ET, reps,
                  xT_in, wqT_in, wkT_in, wvT_in, woT_in, cosT_in, sinT_in,
                  out_d)

    nc.compile()
    return nc


def _emit(tc, nc, S, NQC, ET, reps,
          xT_in, wqT_in, wkT_in, wvT_in, woT_in, cosT_in, sinT_in, out_d):
    from contextlib import ExitStack

    ctx = ExitStack()
    with ctx:
        const = ctx.enter_context(tc.tile_pool(name="const", bufs=1))
        wpool = ctx.enter_context(tc.tile_pool(name="wpool", bufs=1))
        qkv = ctx.enter_context(tc.tile_pool(name="qkv", bufs=1))
        dram = ctx.enter_context(tc.tile_pool(name="dram", bufs=1, space="DRAM"))

        # ---- constants ----
        identf = const.tile([P, P], F32)
        make_identity(nc, identf[:])
        # causal mask for the first 128 q-cols of each trimmed diagonal
        # k-tile: valid(k, q') = (q' - k) >= 0
        maskf = const.tile([P, P], F32)
        nc.gpsimd.memset(maskf[:], 1.0)
        nc.gpsimd.affine_select(
            out=maskf[:], in_=maskf[:],
            compare_op=mybir.AluOpType.is_ge,
            fill=0.0, base=0, pattern=[[1, P]],
            channel_multiplier=-1,
        )
        mask = const.tile([P, P], BF16)
        nc.vector.tensor_copy(mask[:], maskf[:])
        ones_stage = const.tile([P, P], F32)
        nc.gpsimd.memset(ones_stage[:], 1.0)
        ones_red = const.tile([P, 1], F32R)
        nc.vector.tensor_copy(ones_red[:], ones_stage[:, 0:1])
        ones_col = const.tile([1, P], F32R)
        nc.vector.tensor_copy(ones_col[:], ones_stage[0:1, :])

        # ---- persistent SBUF ----
        cosT_sb = wpool.tile([P, S], BF16)
        sinT_sb = wpool.tile([P, S], BF16)
        wqT_sb = wpool.tile([P, ET, DQ], F32R)
        wkT_sb = wpool.tile([P, ET, HD], F32R)
        wvT_sb = wpool.tile([P, ET, HD], F32R)
        woT_sb = wpool.tile([P, ET, DQ], BF16)

        QT_sb = qkv.tile([P, 2, NHL, 512], BF16)   # double-buffered per chunk
        KT_sb = qkv.tile([P, S], BF16)
        V_sb = qkv.tile([P, S // P, HD], BF16)

        # collective bounce buffers (DRAM): full chunks 0..NQC-2, plus two
        # head-pair halves for the last chunk (tail AllGather split)
        agin = [dram.tile([P, NHL * 512], BF16, name=f"agin{c}")
                for c in range(NQC - 1)]
        agout = [dram.tile([4 * P, NHL * 512], BF16, name=f"agout{c}")
                 for c in range(NQC - 1)]
        aginL = [dram.tile([P, 2 * 512], BF16, name=f"aginL{i}")
                 for i in range(2)]
        agoutL = [dram.tile([4 * P, 2 * 512], BF16, name=f"agoutL{i}")
                  for i in range(2)]

        for rep in range(reps):
            _emit_rep(tc, nc, S, NQC, ET, rep,
                      xT_in, wqT_in, wkT_in, wvT_in, woT_in, cosT_in, sinT_in,
                      out_d, identf, mask, ones_red, ones_col,
                      QT_sb, KT_sb, V_sb, agin, agout, aginL, agoutL,
                      cosT_sb, sinT_sb, wqT_sb, wkT_sb, wvT_sb, woT_sb)


def _emit_rep(tc, nc, S, NQC, ET, rep,
              xT_in, wqT_in, wkT_in, wvT_in, woT_in, cosT_in, sinT_in,
              out_d, identf, mask, ones_red, ones_col,
              QT_sb, KT_sb, V_sb, agin, agout, aginL, agoutL,
              cosT_sb, sinT_sb, wqT_sb, wkT_sb, wvT_sb, woT_sb):
    from contextlib import ExitStack

    with ExitStack() as pctx:
        xn_pool = pctx.enter_context(tc.tile_pool(name="xn", bufs=2))
        rope_pool = pctx.enter_context(tc.tile_pool(name="rope", bufs=2))
        vt_pool = pctx.enter_context(tc.tile_pool(name="vt", bufs=2))
        ex_pool = pctx.enter_context(tc.tile_pool(name="ex", bufs=6))
        sm_pool = pctx.enter_context(tc.tile_pool(name="sm", bufs=1))
        dn_pool = pctx.enter_context(tc.tile_pool(name="dn", bufs=2))
        bc_pool = pctx.enter_context(tc.tile_pool(name="bc", bufs=2))
        oh_pool = pctx.enter_context(tc.tile_pool(name="oh", bufs=3))
        af_pool = pctx.enter_context(tc.tile_pool(name="af", bufs=2))
        oc_pool = pctx.enter_context(tc.tile_pool(name="oc", bufs=1))
        pj_ps = pctx.enter_context(tc.tile_pool(name="pj_ps", bufs=2, space="PSUM"))
        sc_ps = pctx.enter_context(tc.tile_pool(name="sc_ps", bufs=2, space="PSUM"))
        pv_ps = pctx.enter_context(tc.tile_pool(name="pv_ps", bufs=1, space="PSUM"))
        dn_ps = pctx.enter_context(tc.tile_pool(name="dn_ps", bufs=1, space="PSUM"))
        bc_ps = pctx.enter_context(tc.tile_pool(name="bc_ps", bufs=1, space="PSUM"))
        oo_ps = pctx.enter_context(tc.tile_pool(name="oo_ps", bufs=1, space="PSUM"))

        x_r = xT_in.rearrange("(et p) s -> p et s", p=P).bitcast(F32R)
        wq_r = wqT_in.rearrange("(et p) d -> p et d", p=P).bitcast(F32R)
        # (whole-tensor loads below use these 3D views directly)
        wk_r = wkT_in.rearrange("(et p) d -> p et d", p=P).bitcast(F32R)
        wv_r = wvT_in.rearrange("(et p) d -> p et d", p=P).bitcast(F32R)
        wo_r = woT_in.rearrange("(et p) d -> p et d", p=P)
        out_r = out_d.rearrange("(ot p) s -> p ot s", p=P)

        def emit_oproj(j, last=False):
            # af loaded as two head-pair halves (h 0-1, then h 2-3) so the
            # split tail AllGather can feed the first half early
            afs = []
            for half in range(2):
                afh = af_pool.tile([P, 4, 2, 512], BF16, name="afh",
                                   tag="afh")
                if last:
                    src = agoutL[half].rearrange(
                        "(r p) (h s) -> p r h s", p=P, h=2)
                else:
                    src = agout[j].rearrange(
                        "(r p) (h s) -> p r h s", p=P,
                        h=NHL)[:, :, 2 * half:2 * half + 2, :]
                nc.sync.dma_start(afh[:], src)
                afs.append(afh)
            ms = ([m for m in range(16) if m % 4 < 2]
                  + [m for m in range(16) if m % 4 >= 2])
            outC = oc_pool.tile([P, 4, 512], F32, name="outC", tag="outC")
            for ot in range(4):
                po = oo_ps.tile([P, 512], F32, name="po", tag="po")
                for i, m in enumerate(ms):
                    r, h = m // 4, m % 4
                    nc.tensor.matmul(
                        po[:], woT_sb[:, m, ot * P:(ot + 1) * P],
                        afs[h // 2][:, r, h % 2, :],
                        start=(i == 0), stop=(i == 15))
                nc.scalar.copy(outC[:, ot, :], po[:])
            nc.sync.dma_start(out_r[:, :, j * 512:(j + 1) * 512], outC[:])

        xts_tiles = {}

        def ensure_x(sc):
            if sc in xts_tiles or sc >= NQC:
                return
            xt = xn_pool.tile([P, ET, 512], F32R, name="xt", tag="xt")
            nc.sync.dma_start(xt[:], x_r[:, :, sc * 512:(sc + 1) * 512])
            xts_tiles[sc] = xt

        for sc in range(NQC):
            s0 = sc * 512
            # ================= projection for chunk sc =================
            if rep == 0 and sc == 0:
                # weights on other queues so x isn't starved
                nc.scalar.dma_start(wqT_sb[:], wq_r[:])
                nc.scalar.dma_start(wkT_sb[:], wk_r[:])
                nc.scalar.dma_start(wvT_sb[:], wv_r[:])
                nc.scalar.dma_start(woT_sb[:], wo_r[:])
                nc.scalar.dma_start(cosT_sb[:], cosT_in[:])
                nc.scalar.dma_start(sinT_sb[:], sinT_in[:])
            ensure_x(sc)
            ensure_x(sc + 1)   # prefetch next chunk (double-buffered pool)
            xts = xts_tiles.pop(sc)

            cos_c = cosT_sb[:, s0:s0 + 512]
            sin_c = sinT_sb[:, s0:s0 + 512]
            for d6 in range(6):
                pp = pj_ps.tile([P, 512], F32, name="pp", tag="pp")
                for et in range(ET):
                    if d6 < 4:
                        lhsT = wqT_sb[:, et, d6 * HD:(d6 + 1) * HD]
                    elif d6 == 4:
                        lhsT = wkT_sb[:, et, :]
                    else:
                        lhsT = wvT_sb[:, et, :]
                    nc.tensor.matmul(pp[:], lhsT, xts[:, et, :],
                                     start=(et == 0), stop=(et == ET - 1))
                if d6 < 5:
                    dst = (QT_sb[:, sc % 2, d6, :] if d6 < 4
                           else KT_sb[:, s0:s0 + 512])
                    t1 = rope_pool.tile([P, 512], BF16, name="t1", tag="t1")
                    t2 = rope_pool.tile([P, 512], BF16, name="t2", tag="t2")
                    nc.vector.tensor_tensor(t1[:], pp[:], cos_c,
                                            mybir.AluOpType.mult)
                    # sinT arrives with rows 0:64 pre-negated (host side)
                    nc.vector.tensor_tensor(t2[0:64, :], pp[64:128, :],
                                            sin_c[0:64, :],
                                            mybir.AluOpType.mult)
                    nc.vector.tensor_tensor(t2[64:128, :], pp[0:64, :],
                                            sin_c[64:128, :],
                                            mybir.AluOpType.mult)
                    nc.vector.tensor_tensor(dst, t1[:], t2[:],
                                            mybir.AluOpType.add)
                else:
                    vts = vt_pool.tile([P, 512], F32, name="vts", tag="vts")
                    nc.scalar.copy(vts[:], pp[:])
                    for st in range(4):
                        pt = pj_ps.tile([P, 512], F32, name="pt",
                                        tag="pp")[:, 0:P]
                        nc.tensor.transpose(pt[:], vts[:, st * P:(st + 1) * P],
                                            identf[:])
                        nc.scalar.copy(V_sb[:, sc * 4 + st, :], pt[:])

            # o_proj for chunk sc-2: two chunks of slack hide the AllGather
            if sc >= 2:
                emit_oproj(sc - 2)

            # ================= attention for chunk sc =================
            qT = QT_sb[:, sc % 2]
            for h in range(NHL):
                nkt = 4 * sc + 4
                pv = pv_ps.tile([P, 512], F32, name="pv", tag="pv")
                pden = dn_ps.tile([1, 512], F32, name="pden", tag="pden")
                dacc = dn_pool.tile([P, 512], F32R, name="dacc", tag="dacc")
                exs = [None] * nkt
                qoffs = [0] * nkt

                def emit_pv(j, last):
                    nc.tensor.matmul(pv[:, qoffs[j]:512], V_sb[:, j, :],
                                     exs[j][:, 0:512 - qoffs[j]],
                                     start=(j == 0), stop=last)

                for kt in range(nkt):
                    t = kt - 4 * sc
                    qoff = 128 * t if t > 0 else 0
                    N = 512 - qoff
                    qoffs[kt] = qoff
                    ps = sc_ps.tile([P, 512], F32, name="ps", tag="ps")
                    nc.tensor.matmul(
                        ps[:, 0:N], KT_sb[:, kt * P:(kt + 1) * P],
                        qT[:, h, qoff:512], start=True, stop=True)
                    ex = ex_pool.tile([P, 512], BF16, name="ex", tag="ex")
                    exs[kt] = ex
                    nc.scalar.activation(ex[:, 0:N], ps[:, 0:N],
                                         mybir.ActivationFunctionType.Exp,
                                         scale=float(SCALE))
                    if t >= 0:
                        # staircase mask on the first 128 trimmed q-cols
                        nc.vector.tensor_tensor(
                            ex[:, 0:P], ex[:, 0:P], mask[:],
                            mybir.AluOpType.mult)
                    # denominator accumulation off the PE (GpSimd is idle)
                    if kt == 0:
                        nc.gpsimd.tensor_copy(dacc[:], ex[:, 0:512])
                    else:
                        nc.gpsimd.tensor_add(dacc[:, qoff:512],
                                             dacc[:, qoff:512].bitcast(F32),
                                             ex[:, 0:N])
                    if kt >= LAG:
                        emit_pv(kt - LAG, last=False)
                for j in range(max(0, nkt - LAG), nkt):
                    emit_pv(j, last=(j == nkt - 1))

                nc.tensor.matmul(pden[0:1, :], ones_red[:], dacc[:],
                                 start=True, stop=True)
                rec = sm_pool.tile([1, 512], F32, name="rec", tag="rec")
                nc.vector.reciprocal_approx_fast(out=rec[:], in_=pden[0:1, :])
                rec_r = sm_pool.tile([1, 512], F32R, name="rec_r", tag="rec_r")
                nc.vector.tensor_copy(rec_r[:], rec[:])
                pbc = bc_ps.tile([P, 512], F32, name="pbc", tag="pbc")
                nc.tensor.matmul(pbc[:], ones_col[:], rec_r[:],
                                 start=True, stop=True)
                bcr = bc_pool.tile([P, 512], BF16, name="bcr", tag="bcr")
                nc.scalar.copy(bcr[:], pbc[:])
                outH = oh_pool.tile([P, 512], BF16, name="outH", tag="outH")
                nc.vector.tensor_tensor(outH[:], pv[:], bcr[:],
                                        mybir.AluOpType.mult)
                if sc < NQC - 1:
                    nc.sync.dma_start(agin[sc][:, h * 512:(h + 1) * 512],
                                      outH[:])
                else:
                    nc.sync.dma_start(
                        aginL[h // 2][:, (h % 2) * 512:(h % 2 + 1) * 512],
                        outH[:])
                    if h % 2 == 1:
                        nc.gpsimd.collective_compute(
                            "AllGather", mybir.AluOpType.bypass,
                            replica_groups=REPLICA_GROUPS,
                            ins=[aginL[h // 2].opt()],
                            outs=[agoutL[h // 2].opt()])

            # ---- ship chunk sc: AllGather across the batch group ----
            if sc < NQC - 1:
                nc.gpsimd.collective_compute(
                    "AllGather", mybir.AluOpType.bypass,
                    replica_groups=REPLICA_GROUPS,
                    ins=[agin[sc].opt()],
                    outs=[agout[sc].opt()])
        if NQC >= 2:
            emit_oproj(NQC - 2)
        emit_oproj(NQC - 1, last=True)


# ======================= host side =======================

_CACHE = {}


def _get_program(S=2048, reps=1):
    key = (S, reps)
    if key not in _CACHE:
        _CACHE[key] = build_program(S=S, reps=reps)
    return _CACHE[key]


def make_in_maps(x, cos, sin, wq, wk, wv, wo):
    in_maps = []
    cosT = np.ascontiguousarray(cos.T.astype(NPBF16))
    sinT = sin.T.astype(np.float32).copy()
    sinT[:HD // 2, :] *= -1.0   # fold rotate_half sign into the table
    sinT = np.ascontiguousarray(sinT.astype(NPBF16))
    for c in range(8):
        b, g = c // 4, c % 4
        in_maps.append({
            "xT": np.ascontiguousarray(x[b].T.astype(np.float32)),
            "wqT": np.ascontiguousarray(wq[g * DQ:(g + 1) * DQ, :].T.astype(np.float32)),
            "wkT": np.ascontiguousarray(wk[g * HD:(g + 1) * HD, :].T.astype(np.float32)),
            "wvT": np.ascontiguousarray(wv[g * HD:(g + 1) * HD, :].T.astype(np.float32)),
            "woT": np.ascontiguousarray(wo[g * DQ:(g + 1) * DQ, :].T.astype(NPBF16)),
            "cosT": cosT,
            "sinT": sinT,
        })
    return in_maps


def assemble_output(results, B, S):
    out = np.empty((B, S, E), np.float32)
    for c in range(8):
        b, g = c // 4, c % 4
        out[b][:, g * DQ:(g + 1) * DQ] = results[c]["out"].T
    return out


# ---- inline SPMD runner (PJRT/axon), device-resident inputs ----

class SpmdRunner:
    def __init__(self, nc, n_cores):
        import jax
        from jax.sharding import Mesh, PartitionSpec
        from jax.experimental.shard_map import shard_map
        from concourse import bass2jax
        from concourse.bass2jax import _bass_exec_p, install_neuronx_cc_hook

        install_neuronx_cc_hook()
        self.jax = jax
        self.nc = nc
        self.n_cores = n_cores
        partition_name = (nc.partition_id_tensor.name
                          if nc.partition_id_tensor else None)
        in_names, out_names, out_avals = [], [], []
        zero_outs = []
        for alloc in nc.m.functions[0].allocations:
            if not isinstance(alloc, mybir.MemoryLocationSet):
                continue
            name = alloc.memorylocations[0].name
            if alloc.kind == "ExternalInput":
                if name != partition_name:
                    in_names.append(name)
            elif alloc.kind == "ExternalOutput":
                out_names.append(name)
                shape = tuple(alloc.tensor_shape)
                dtype = mybir.dt.np(alloc.dtype)
                out_avals.append(jax.core.ShapedArray(shape, dtype))
                zero_outs.append(np.zeros(shape, dtype))
        self.in_names, self.out_names = in_names, out_names
        self.out_avals, self.zero_outs = out_avals, zero_outs
        self.n_params = len(in_names)

        all_in = list(in_names) + list(out_names)
        if partition_name is not None:
            all_in.append(partition_name)

        def _body(*args):
            operands = list(args)
            if partition_name is not None:
                operands.append(bass2jax.partition_id_tensor())
            outs = _bass_exec_p.bind(
                *operands, out_avals=tuple(out_avals),
                in_names=tuple(all_in), out_names=tuple(out_names),
                lowering_input_output_aliases=(),
                sim_require_finite=True, sim_require_nnan=True, nc=nc)
            return tuple(outs)

        devices = jax.devices()[:n_cores]
        self.mesh = Mesh(np.asarray(devices), ("core",))
        n_outs = len(out_names)
        in_specs = (PartitionSpec("core"),) * (self.n_params + n_outs)
        out_specs = (PartitionSpec("core"),) * n_outs
        self.fn = jax.jit(
            shard_map(_body, mesh=self.mesh, in_specs=in_specs,
                      out_specs=out_specs, check_rep=False),
            keep_unused=True)
        self.dev_args = None

    def stage_inputs(self, in_maps):
        import jax
        from jax.sharding import PartitionSpec
        per_core = [[np.asarray(m[n]) for n in self.in_names] for m in in_maps]
        concat_in = [
            np.concatenate([per_core[c][i] for c in range(self.n_cores)], axis=0)
            for i in range(self.n_params)]
        concat_zeros = [
            np.zeros((self.n_cores * z.shape[0], *z.shape[1:]), z.dtype)
            for z in self.zero_outs]
        sharding = jax.sharding.NamedSharding(self.mesh, PartitionSpec("core"))
        self.dev_args = [jax.device_put(a, sharding)
                         for a in (*concat_in, *concat_zeros)]
        for a in self.dev_args:
            a.block_until_ready()

    def run(self):
        out_arrs = [np.asarray(o) for o in self.fn(*self.dev_args)]
        return [
            {n: out_arrs[i].reshape(self.n_cores, *self.out_avals[i].shape)[c]
             for i, n in enumerate(self.out_names)}
            for c in range(self.n_cores)]

    def time_exec(self, iters=30, warmup=3):
        import jax
        for _ in range(warmup):
            res = self.fn(*self.dev_args)
        jax.block_until_ready(res)
        t0 = time.perf_counter()
        for _ in range(iters):
            res = self.fn(*self.dev_args)
        jax.block_until_ready(res)
        t1 = time.perf_counter()
        return (t1 - t0) / iters * 1e9


_RUNNER_CACHE = {}


def get_runner(S=2048, reps=1):
    key = (S, reps)
    if key not in _RUNNER_CACHE:
        nc = _get_program(S=S, reps=reps)
        _RUNNER_CACHE[key] = SpmdRunner(nc, 8)
    return _RUNNER_CACHE[key]


def kernel(x, cos, sin, wq, wk, wv, wo):
    B, S, _ = x.shape
    runner = get_runner(S=S, reps=1)
    runner.stage_inputs(make_in_maps(x, cos, sin, wq, wk, wv, wo))
    results = runner.run()
    return assemble_output(results, B, S)


if __name__ == "__main__":
    # tiny self-test against a local numpy reference
    S = int(sys.argv[1]) if len(sys.argv) > 1 else 512
    rng = np.random.default_rng(0)
    B, H, HKV = 2, 16, 4
    x = rng.standard_normal((B, S, E), dtype=np.float32)
    cos = rng.random((S, HD), dtype=np.float32)
    sin = rng.random((S, HD), dtype=np.float32)
    sc = 0.02
    wq = (rng.standard_normal((H * HD, E), dtype=np.float32) * sc)
    wk = (rng.standard_normal((HKV * HD, E), dtype=np.float32) * sc)
    wv = (rng.standard_normal((HKV * HD, E), dtype=np.float32) * sc)
    wo = (rng.standard_normal((E, H * HD), dtype=np.float32) * sc)

    def ref(x, cos, sin, wq, wk, wv, wo):
        x64 = x.astype(np.float64)
        q = (x64 @ wq.T.astype(np.float64)).reshape(B, S, H, HD)
        k = (x64 @ wk.T.astype(np.float64)).reshape(B, S, HKV, HD)
        v = (x64 @ wv.T.astype(np.float64)).reshape(B, S, HKV, HD)

        def rot(t):
            return np.concatenate([-t[..., HD // 2:], t[..., :HD // 2]], -1)

        c = cos[:, None, :].astype(np.float64)
        s = sin[:, None, :].astype(np.float64)
        q = q * c + rot(q) * s
        k = k * c + rot(k) * s
        k = np.repeat(k, H // HKV, axis=2).transpose(0, 2, 1, 3)
        v = np.repeat(v, H // HKV, axis=2).transpose(0, 2, 1, 3)
        q = q.transpose(0, 2, 1, 3)
        scores = np.einsum("bhqd,bhkd->bhqk", q, k) / np.sqrt(HD)
        mask = np.tril(np.ones((S, S), bool))
        scores = np.where(mask, scores, -np.inf)
        scores -= scores.max(-1, keepdims=True)
        p = np.exp(scores)
        p /= p.sum(-1, keepdims=True)
        o = np.einsum("bhqk,bhkd->bhqd", p, v)
        o = o.transpose(0, 2, 1, 3).reshape(B, S, H * HD)
        return o @ wo.T.astype(np.float64)

    want = ref(x, cos, sin, wq, wk, wv, wo)
    got = kernel(x, cos, sin, wq, wk, wv, wo)
    err = np.abs(got - want).max() / np.abs(want).max()
    print(f"S={S}: rel err (absmax-relative) = {err:.3e}")


# revision 25
# speedup vs baseline: 1.5040x; 1.5040x over previous
"""Trainium2 Bass kernel for nn_Attention (dense transformer block:
QKV proj + RoPE + causal GQA attention + o_proj), SPMD over 8 NeuronCores.

Sharding: core c -> (batch b = c//4, head-group g = c%4). Each core computes
4 query heads + its kv head for one batch; per s-chunk the 4 head outputs are
AllGather'd (bf16) within the 4-core batch group and each core computes a
disjoint 512-column slice of the o_proj output for that chunk.

v2 vs v1:
- x passed host-transposed (xT [E,S]) -> no PE transposes / PSUM evacuation.
- chunk-outer pipeline: proj(sc) -> attention(sc, 4 heads) -> AllGather(sc)
  -> o_proj(sc-1), so collectives overlap the next chunk's compute.
- bf16 for QT/KT/V/probs/AllGather/o_proj operands (PE rate unchanged,
  halves collective bytes, 2x DVE mask, FWL weight loads).
- causal trim: diagonal k-tiles only compute q-columns >= 128*t.
- o_proj accumulates all 16 head-blocks in PSUM (no DVE adds / outAcc).
- reciprocal_approx_fast instead of DVE reciprocal (3.3us -> ~0.7us).
"""

import sys
import time

sys.path.insert(0, "/opt/trn_rl_repo")

import numpy as np
import ml_dtypes

import concourse.bass as bass
import concourse.mybir as mybir
import concourse.tile as tile
from concourse import bacc
from concourse.masks import make_identity

F32 = mybir.dt.float32
F32R = mybir.dt.float32r
BF16 = mybir.dt.bfloat16
NPBF16 = ml_dtypes.bfloat16
P = 128
HD = 128            # head dim
NHL = 4             # query heads per core
E = 2048            # hidden
DQ = NHL * HD       # 512, local q-projection width / o-slice width
SCALE = 1.0 / np.sqrt(np.float32(HD))
REPLICA_GROUPS = [[0, 1, 2, 3], [4, 5, 6, 7]]
LAG = 4             # pv(kt-LAG) emitted after scores(kt): hides exp+mask


def r32(ap):
    return ap.bitcast(F32R)


def build_program(S=2048, reps=1, n_cores=8):
    """Build the per-core SPMD Bass program. Returns compiled nc."""
    NQC = S // 512       # 512-wide chunks along sequence
    ET = E // P          # 16 tiles along hidden

    nc = bacc.Bacc("TRN2", target_bir_lowering=False, debug=False,
                   num_devices=n_cores)

    xT_in = nc.declare_dram_parameter("xT", [E, S], BF16, isOutput=False)
    wqT_in = nc.declare_dram_parameter("wqT", [E, DQ], BF16, isOutput=False)
    wkT_in = nc.declare_dram_parameter("wkT", [E, HD], BF16, isOutput=False)
    wvT_in = nc.declare_dram_parameter("wvT", [E, HD], BF16, isOutput=False)
    woT_in = nc.declare_dram_parameter("woT", [E, DQ], BF16, isOutput=False)
    cosT_in = nc.declare_dram_parameter("cosT", [HD, S], BF16, isOutput=False)
    sinT_in = nc.declare_dram_parameter("sinT", [HD, S], BF16, isOutput=False)
    out_d = nc.declare_dram_parameter("out", [DQ, S], F32, isOutput=True)

    with tile.TileContext(nc) as tc:
        with nc.allow_low_precision(reason="bf16/f32r attention pipeline"):
            _emit(tc, nc, S, NQC, ET, reps,
                  xT_in, wqT_in, wkT_in, wvT_in, woT_in, cosT_in, sinT_in,
                  out_d)

    nc.compile()
    return nc


def _emit(tc, nc, S, NQC, ET, reps,
          xT_in, wqT_in, wkT_in, wvT_in, woT_in, cosT_in, sinT_in, out_d):
    from contextlib import ExitStack

    ctx = ExitStack()
    with ctx:
        const = ctx.enter_context(tc.tile_pool(name="const", bufs=1))
        wpool = ctx.enter_context(tc.tile_pool(name="wpool", bufs=1))
        qkv = ctx.enter_context(tc.tile_pool(name="qkv", bufs=1))
        dram = ctx.enter_context(tc.tile_pool(name="dram", bufs=1, space="DRAM"))
        xn_pool = ctx.enter_context(tc.tile_pool(name="xn", bufs=2))
        rope_pool = ctx.enter_context(tc.tile_pool(name="rope", bufs=2))
        vt_pool = ctx.enter_context(tc.tile_pool(name="vt", bufs=2))
        ex_pool = ctx.enter_context(tc.tile_pool(name="ex", bufs=6))
        sm_pool = ctx.enter_context(tc.tile_pool(name="sm", bufs=1))
        dn_pool = ctx.enter_context(tc.tile_pool(name="dn", bufs=2))
        bc_pool = ctx.enter_context(tc.tile_pool(name="bc", bufs=2))
        oh_pool = ctx.enter_context(tc.tile_pool(name="oh", bufs=3))
        af_pool = ctx.enter_context(tc.tile_pool(name="af", bufs=2))
        oc_pool = ctx.enter_context(tc.tile_pool(name="oc", bufs=1))
        pj_ps = ctx.enter_context(tc.tile_pool(name="pj_ps", bufs=2, space="PSUM"))
        sc_ps = ctx.enter_context(tc.tile_pool(name="sc_ps", bufs=3, space="PSUM"))
        pv_ps = ctx.enter_context(tc.tile_pool(name="pv_ps", bufs=1, space="PSUM"))
        dn_ps = ctx.enter_context(tc.tile_pool(name="dn_ps", bufs=1, space="PSUM"))
        oo_ps = ctx.enter_context(tc.tile_pool(name="oo_ps", bufs=1, space="PSUM"))

        # ---- constants ----
        identf = const.tile([P, P], F32)
        make_identity(nc, identf[:])
        # causal mask for the first 128 q-cols of each trimmed diagonal
        # k-tile: valid(k, q') = (q' - k) >= 0
        maskf = const.tile([P, P], F32)
        nc.gpsimd.memset(maskf[:], 1.0)
        nc.gpsimd.affine_select(
            out=maskf[:], in_=maskf[:],
            compare_op=mybir.AluOpType.is_ge,
            fill=0.0, base=0, pattern=[[1, P]],
            channel_multiplier=-1,
        )
        mask = const.tile([P, P], BF16)
        nc.vector.tensor_copy(mask[:], maskf[:])
        ones_stage = const.tile([P, P], F32)
        nc.gpsimd.memset(ones_stage[:], 1.0)
        ones_red = const.tile([P, 1], BF16)
        nc.vector.tensor_copy(ones_red[:], ones_stage[:, 0:1])
        ones_col = const.tile([1, P], F32R)
        nc.vector.tensor_copy(ones_col[:], ones_stage[0:1, :])

        # ---- persistent SBUF ----
        cosT_sb = wpool.tile([P, S], BF16)
        sinT_sb = wpool.tile([P, S], BF16)
        wqT_sb = wpool.tile([P, ET, DQ], BF16)
        wkT_sb = wpool.tile([P, ET, HD], BF16)
        wvT_sb = wpool.tile([P, ET, HD], BF16)
        woT_sb = wpool.tile([P, ET, DQ], BF16)

        QT_sb = qkv.tile([P, 2, NHL, 512], BF16)   # double-buffered per chunk
        KT_sb = qkv.tile([P, S], BF16)
        V_sb = qkv.tile([P, S // P, HD], BF16)

        # collective bounce buffers (DRAM): full chunks 0..NQC-2, plus two
        # head-pair halves for the last chunk (tail AllGather split)
        agin = [dram.tile([P, NHL * 512], BF16, name=f"agin{c}")
                for c in range(NQC - 1)]
        agout = [dram.tile([4 * P, NHL * 512], BF16, name=f"agout{c}")
                 for c in range(NQC - 1)]
        aginL = [dram.tile([P, 2 * 512], BF16, name=f"aginL{i}")
                 for i in range(2)]
        agoutL = [dram.tile([4 * P, 2 * 512], BF16, name=f"agoutL{i}")
                  for i in range(2)]

        x_r = xT_in.rearrange("(et p) s -> p et s", p=P)
        wq_r = wqT_in.rearrange("(et p) d -> p et d", p=P)
        wk_r = wkT_in.rearrange("(et p) d -> p et d", p=P)
        wv_r = wvT_in.rearrange("(et p) d -> p et d", p=P)
        wo_r = woT_in.rearrange("(et p) d -> p et d", p=P)
        out_r = out_d.rearrange("(ot p) s -> p ot s", p=P)

        def emit_oproj(g):
            sc = g % NQC
            last = (sc == NQC - 1)
            # af loaded as two head-pair halves (h 0-1, then h 2-3) so the
            # split tail AllGather can feed the first half early
            afs = []
            for half in range(2):
                afh = af_pool.tile([P, 4, 2, 512], BF16, name="afh",
                                   tag="afh")
                if last:
                    src = agoutL[half].rearrange(
                        "(r p) (h s) -> p r h s", p=P, h=2)
                else:
                    src = agout[sc].rearrange(
                        "(r p) (h s) -> p r h s", p=P,
                        h=NHL)[:, :, 2 * half:2 * half + 2, :]
                nc.sync.dma_start(afh[:], src)
                afs.append(afh)
            ms = ([m for m in range(16) if m % 4 < 2]
                  + [m for m in range(16) if m % 4 >= 2])
            outC = oc_pool.tile([P, 4, 512], F32, name="outC", tag="outC")
            for ot in range(4):
                po = oo_ps.tile([P, 512], F32, name="po", tag="po")
                for i, m in enumerate(ms):
                    r, h = m // 4, m % 4
                    nc.tensor.matmul(
                        po[:], woT_sb[:, m, ot * P:(ot + 1) * P],
                        afs[h // 2][:, r, h % 2, :],
                        start=(i == 0), stop=(i == 15))
                nc.scalar.copy(outC[:, ot, :], po[:])
            nc.sync.dma_start(out_r[:, :, sc * 512:(sc + 1) * 512], outC[:])

        xts_tiles = {}

        def ensure_x(g):
            if g in xts_tiles or g >= reps * NQC:
                return
            xt = xn_pool.tile([P, ET, 512], BF16, name="xt", tag="xt")
            sc = g % NQC
            nc.sync.dma_start(xt[:], x_r[:, :, sc * 512:(sc + 1) * 512])
            xts_tiles[g] = xt

        def load_weights():
            nc.scalar.dma_start(wqT_sb[:], wq_r[:])
            nc.scalar.dma_start(wkT_sb[:], wk_r[:])
            nc.scalar.dma_start(wvT_sb[:], wv_r[:])
            nc.scalar.dma_start(woT_sb[:], wo_r[:])
            nc.scalar.dma_start(cosT_sb[:], cosT_in[:])
            nc.scalar.dma_start(sinT_sb[:], sinT_in[:])

        load_weights()
        G = reps * NQC
        for g in range(G):
            rep, sc = divmod(g, NQC)
            s0 = sc * 512
            # ================= projection for chunk sc =================
            ensure_x(g)
            ensure_x(g + 1)   # prefetch next chunk (double-buffered pool)
            xts = xts_tiles.pop(g)

            cos_c = cosT_sb[:, s0:s0 + 512]
            sin_c = sinT_sb[:, s0:s0 + 512]
            for d6 in range(6):
                pp = pj_ps.tile([P, 512], F32, name="pp", tag="pp")
                for et in range(ET):
                    if d6 < 4:
                        lhsT = wqT_sb[:, et, d6 * HD:(d6 + 1) * HD]
                    elif d6 == 4:
                        lhsT = wkT_sb[:, et, :]
                    else:
                        lhsT = wvT_sb[:, et, :]
                    nc.tensor.matmul(pp[:], lhsT, xts[:, et, :],
                                     start=(et == 0), stop=(et == ET - 1))
                if d6 < 5:
                    dst = (QT_sb[:, g % 2, d6, :] if d6 < 4
                           else KT_sb[:, s0:s0 + 512])
                    t1 = rope_pool.tile([P, 512], BF16, name="t1", tag="t1")
                    t2 = rope_pool.tile([P, 512], BF16, name="t2", tag="t2")
                    nc.vector.tensor_tensor(t1[:], pp[:], cos_c,
                                            mybir.AluOpType.mult)
                    # sinT arrives with rows 0:64 pre-negated (host side)
                    nc.vector.tensor_tensor(t2[0:64, :], pp[64:128, :],
                                            sin_c[0:64, :],
                                            mybir.AluOpType.mult)
                    nc.vector.tensor_tensor(t2[64:128, :], pp[0:64, :],
                                            sin_c[64:128, :],
                                            mybir.AluOpType.mult)
                    nc.vector.tensor_tensor(dst, t1[:], t2[:],
                                            mybir.AluOpType.add)
                else:
                    vts = vt_pool.tile([P, 512], F32, name="vts", tag="vts")
                    nc.scalar.copy(vts[:], pp[:])
                    for st in range(4):
                        pt = pj_ps.tile([P, 512], F32, name="pt",
                                        tag="pp")[:, 0:P]
                        nc.tensor.transpose(pt[:], vts[:, st * P:(st + 1) * P],
                                            identf[:])
                        nc.scalar.copy(V_sb[:, sc * 4 + st, :], pt[:])

            # o_proj two global chunks back: the AllGather has had a full
            # chunk of compute to complete, and rep boundaries pipeline
            if g >= 2:
                emit_oproj(g - 2)

            # ================= attention for chunk sc =================
            qT = QT_sb[:, g % 2]
            for h in range(NHL):
                nkt = 4 * sc + 4
                pv = pv_ps.tile([P, 512], F32, name="pv", tag="pv")
                pden = dn_ps.tile([P, 512], F32, name="pden",
                                  tag="pden")[0:1, :]
                dacc = dn_pool.tile([P, 512], BF16, name="dacc", tag="dacc")
                exs = [None] * nkt
                qoffs = [0] * nkt

                def emit_pv(j, last):
                    nc.tensor.matmul(pv[:, qoffs[j]:512], V_sb[:, j, :],
                                     exs[j][:, 0:512 - qoffs[j]],
                                     start=(j == 0), stop=last)

                for kt in range(nkt):
                    t = kt - 4 * sc
                    qoff = 128 * t if t > 0 else 0
                    N = 512 - qoff
                    qoffs[kt] = qoff
                    ps = sc_ps.tile([P, 512], F32, name="ps", tag="ps")
                    nc.tensor.matmul(
                        ps[:, 0:N], KT_sb[:, kt * P:(kt + 1) * P],
                        qT[:, h, qoff:512], start=True, stop=True)
                    ex = ex_pool.tile([P, 512], BF16, name="ex", tag="ex")
                    exs[kt] = ex
                    nc.scalar.activation(ex[:, 0:N], ps[:, 0:N],
                                         mybir.ActivationFunctionType.Exp,
                                         scale=float(SCALE))
                    if t >= 0:
                        # staircase mask on the first 128 trimmed q-cols
                        nc.vector.tensor_tensor(
                            ex[:, 0:P], ex[:, 0:P], mask[:],
                            mybir.AluOpType.mult)
                    # denominator accumulation off the PE (DVE bf16 2x)
                    if kt == 0:
                        nc.vector.tensor_copy(dacc[:], ex[:, 0:512])
                    else:
                        nc.vector.tensor_add(dacc[:, qoff:512],
                                             dacc[:, qoff:512],
                                             ex[:, 0:N])
                    if kt >= LAG:
                        emit_pv(kt - LAG, last=False)
                for j in range(max(0, nkt - LAG), nkt):
                    emit_pv(j, last=(j == nkt - 1))

                nc.tensor.matmul(pden[:], ones_red[:], dacc[:],
                                 start=True, stop=True)
                rec = sm_pool.tile([1, 512], F32, name="rec", tag="rec")
                nc.vector.reciprocal_approx_fast(out=rec[:], in_=pden[:])
                rec_r = sm_pool.tile([1, 512], F32R, name="rec_r", tag="rec_r")
                nc.vector.tensor_copy(rec_r[:], rec[:])
                pbc = dn_ps.tile([P, 512], F32, name="pbc", tag="pden")
                nc.tensor.matmul(pbc[:], ones_col[:], rec_r[:],
                                 start=True, stop=True)
                bcr = bc_pool.tile([P, 512], BF16, name="bcr", tag="bcr")
                nc.scalar.copy(bcr[:], pbc[:])
                outH = oh_pool.tile([P, 512], BF16, name="outH", tag="outH")
                nc.vector.tensor_tensor(outH[:], pv[:], bcr[:],
                                        mybir.AluOpType.mult)
                if sc < NQC - 1:
                    nc.sync.dma_start(agin[sc][:, h * 512:(h + 1) * 512],
                                      outH[:])
                else:
                    nc.sync.dma_start(
                        aginL[h // 2][:, (h % 2) * 512:(h % 2 + 1) * 512],
                        outH[:])
                    if h % 2 == 1:
                        nc.gpsimd.collective_compute(
                            "AllGather", mybir.AluOpType.bypass,
                            replica_groups=REPLICA_GROUPS,
                            ins=[aginL[h // 2].opt()],
                            outs=[agoutL[h // 2].opt()])

            if sc == NQC - 1 and rep < reps - 1:
                load_weights()
            # ---- ship chunk sc: AllGather across the batch group ----
            if sc < NQC - 1:
                nc.gpsimd.collective_compute(
                    "AllGather", mybir.AluOpType.bypass,
                    replica_groups=REPLICA_GROUPS,
                    ins=[agin[sc].opt()],
                    outs=[agout[sc].opt()])
        if G >= 2:
            emit_oproj(G - 2)
        emit_oproj(G - 1)


# ======================= host side =======================

_CACHE = {}


def _get_program(S=2048, reps=1):
    key = (S, reps)
    if key not in _CACHE:
        _CACHE[key] = build_program(S=S, reps=reps)
    return _CACHE[key]


def make_in_maps(x, cos, sin, wq, wk, wv, wo):
    in_maps = []
    cosT = np.ascontiguousarray(cos.T.astype(NPBF16))
    sinT = sin.T.astype(np.float32).copy()
    sinT[:HD // 2, :] *= -1.0   # fold rotate_half sign into the table
    sinT = np.ascontiguousarray(sinT.astype(NPBF16))
    for c in range(8):
        b, g = c // 4, c % 4
        in_maps.append({
            "xT": np.ascontiguousarray(x[b].T.astype(NPBF16)),
            "wqT": np.ascontiguousarray(wq[g * DQ:(g + 1) * DQ, :].T.astype(NPBF16)),
            "wkT": np.ascontiguousarray(wk[g * HD:(g + 1) * HD, :].T.astype(NPBF16)),
            "wvT": np.ascontiguousarray(wv[g * HD:(g + 1) * HD, :].T.astype(NPBF16)),
            "woT": np.ascontiguousarray(wo[g * DQ:(g + 1) * DQ, :].T.astype(NPBF16)),
            "cosT": cosT,
            "sinT": sinT,
        })
    return in_maps


def assemble_output(results, B, S):
    out = np.empty((B, S, E), np.float32)
    for c in range(8):
        b, g = c // 4, c % 4
        out[b][:, g * DQ:(g + 1) * DQ] = results[c]["out"].T
    return out


# ---- inline SPMD runner (PJRT/axon), device-resident inputs ----

class SpmdRunner:
    def __init__(self, nc, n_cores):
        import jax
        from jax.sharding import Mesh, PartitionSpec
        from jax.experimental.shard_map import shard_map
        from concourse import bass2jax
        from concourse.bass2jax import _bass_exec_p, install_neuronx_cc_hook

        install_neuronx_cc_hook()
        self.jax = jax
        self.nc = nc
        self.n_cores = n_cores
        partition_name = (nc.partition_id_tensor.name
                          if nc.partition_id_tensor else None)
        in_names, out_names, out_avals = [], [], []
        zero_outs = []
        for alloc in nc.m.functions[0].allocations:
            if not isinstance(alloc, mybir.MemoryLocationSet):
                continue
            name = alloc.memorylocations[0].name
            if alloc.kind == "ExternalInput":
                if name != partition_name:
                    in_names.append(name)
            elif alloc.kind == "ExternalOutput":
                out_names.append(name)
                shape = tuple(alloc.tensor_shape)
                dtype = mybir.dt.np(alloc.dtype)
                out_avals.append(jax.core.ShapedArray(shape, dtype))
                zero_outs.append(np.zeros(shape, dtype))
        self.in_names, self.out_names = in_names, out_names
        self.out_avals, self.zero_outs = out_avals, zero_outs
        self.n_params = len(in_names)

        all_in = list(in_names) + list(out_names)
        if partition_name is not None:
            all_in.append(partition_name)

        def _body(*args):
            operands = list(args)
            if partition_name is not None:
                operands.append(bass2jax.partition_id_tensor())
            outs = _bass_exec_p.bind(
                *operands, out_avals=tuple(out_avals),
                in_names=tuple(all_in), out_names=tuple(out_names),
                lowering_input_output_aliases=(),
                sim_require_finite=True, sim_require_nnan=True, nc=nc)
            return tuple(outs)

        devices = jax.devices()[:n_cores]
        self.mesh = Mesh(np.asarray(devices), ("core",))
        n_outs = len(out_names)
        in_specs = (PartitionSpec("core"),) * (self.n_params + n_outs)
        out_specs = (PartitionSpec("core"),) * n_outs
        self.fn = jax.jit(
            shard_map(_body, mesh=self.mesh, in_specs=in_specs,
                      out_specs=out_specs, check_rep=False),
            keep_unused=True)
        self.dev_args = None

    def stage_inputs(self, in_maps):
        import jax
        from jax.sharding import PartitionSpec
        per_core = [[np.asarray(m[n]) for n in self.in_names] for m in in_maps]
        concat_in = [
            np.concatenate([per_core[c][i] for c in range(self.n_cores)], axis=0)
            for i in range(self.n_params)]
        concat_zeros = [
            np.zeros((self.n_cores * z.shape[0], *z.shape[1:]), z.dtype)
            for z in self.zero_outs]
        sharding = jax.sharding.NamedSharding(self.mesh, PartitionSpec("core"))
        self.dev_args = [jax.device_put(a, sharding)
                         for a in (*concat_in, *concat_zeros)]
        for a in self.dev_args:
            a.block_until_ready()

    def run(self):
        out_arrs = [np.asarray(o) for o in self.fn(*self.dev_args)]
        return [
            {n: out_arrs[i].reshape(self.n_cores, *self.out_avals[i].shape)[c]
             for i, n in enumerate(self.out_names)}
            for c in range(self.n_cores)]

    def time_exec(self, iters=30, warmup=3):
        import jax
        for _ in range(warmup):
            res = self.fn(*self.dev_args)
        jax.block_until_ready(res)
        t0 = time.perf_counter()
        for _ in range(iters):
            res = self.fn(*self.dev_args)
        jax.block_until_ready(res)
        t1 = time.perf_counter()
        return (t1 - t0) / iters * 1e9


_RUNNER_CACHE = {}


def get_runner(S=2048, reps=1):
    key = (S, reps)
    if key not in _RUNNER_CACHE:
        nc = _get_program(S=S, reps=reps)
        _RUNNER_CACHE[key] = SpmdRunner(nc, 8)
    return _RUNNER_CACHE[key]


def kernel(x, cos, sin, wq, wk, wv, wo):
    B, S, _ = x.shape
    runner = get_runner(S=S, reps=1)
    runner.stage_inputs(make_in_maps(x, cos, sin, wq, wk, wv, wo))
    results = runner.run()
    return assemble_output(results, B, S)


if __name__ == "__main__":
    # tiny self-test against a local numpy reference
    S = int(sys.argv[1]) if len(sys.argv) > 1 else 512
    rng = np.random.default_rng(0)
    B, H, HKV = 2, 16, 4
    x = rng.standard_normal((B, S, E), dtype=np.float32)
    cos = rng.random((S, HD), dtype=np.float32)
    sin = rng.random((S, HD), dtype=np.float32)
    sc = 0.02
    wq = (rng.standard_normal((H * HD, E), dtype=np.float32) * sc)
    wk = (rng.standard_normal((HKV * HD, E), dtype=np.float32) * sc)
    wv = (rng.standard_normal((HKV * HD, E), dtype=np.float32) * sc)
    wo = (rng.standard_normal((E, H * HD), dtype=np.float32) * sc)

    def ref(x, cos, sin, wq, wk, wv, wo):
        x64 = x.astype(np.float64)
        q = (x64 @ wq.T.astype(np.float64)).reshape(B, S, H, HD)
        k = (x64 @ wk.T.astype(np.float64)).reshape(B, S, HKV, HD)
        v = (x64 @ wv.T.astype(np.float64)).reshape(B, S, HKV, HD)

        def rot(t):
            return np.concatenate([-t[..., HD // 2:], t[..., :HD // 2]], -1)

        c = cos[:, None, :].astype(np.float64)
        s = sin[:, None, :].astype(np.float64)
        q = q * c + rot(q) * s
        k = k * c + rot(k) * s
        k = np.repeat(k, H // HKV, axis=2).transpose(0, 2, 1, 3)
        v = np.repeat(v, H // HKV, axis=2).transpose(0, 2, 1, 3)
        q = q.transpose(0, 2, 1, 3)
        scores = np.einsum("bhqd,bhkd->bhqk", q, k) / np.sqrt(HD)
        mask = np.tril(np.ones((S, S), bool))
        scores = np.where(mask, scores, -np.inf)
        scores -= scores.max(-1, keepdims=True)
        p = np.exp(scores)
        p /= p.sum(-1, keepdims=True)
        o = np.einsum("bhqk,bhkd->bhqd", p, v)
        o = o.transpose(0, 2, 1, 3).reshape(B, S, H * HD)
        return o @ wo.T.astype(np.float64)

    want = ref(x, cos, sin, wq, wk, wv, wo)
    got = kernel(x, cos, sin, wq, wk, wv, wo)
    err = np.abs(got - want).max() / np.abs(want).max()
    print(f"S={S}: rel err (absmax-relative) = {err:.3e}")


# revision 27
# speedup vs baseline: 1.5893x; 1.0567x over previous
"""Trainium2 Bass kernel for nn_Attention (dense transformer block:
QKV proj + RoPE + causal GQA attention + o_proj), SPMD over 8 NeuronCores.

Sharding: core c -> (batch b = c//4, head-group g = c%4). Each core computes
4 query heads + its kv head for one batch; per s-chunk the 4 head outputs are
AllGather'd (bf16) within the 4-core batch group and each core computes a
disjoint 512-column slice of the o_proj output for that chunk.

v2 vs v1:
- x passed host-transposed (xT [E,S]) -> no PE transposes / PSUM evacuation.
- chunk-outer pipeline: proj(sc) -> attention(sc, 4 heads) -> AllGather(sc)
  -> o_proj(sc-1), so collectives overlap the next chunk's compute.
- bf16 for QT/KT/V/probs/AllGather/o_proj operands (PE rate unchanged,
  halves collective bytes, 2x DVE mask, FWL weight loads).
- causal trim: diagonal k-tiles only compute q-columns >= 128*t.
- o_proj accumulates all 16 head-blocks in PSUM (no DVE adds / outAcc).
- reciprocal_approx_fast instead of DVE reciprocal (3.3us -> ~0.7us).
"""

import sys
import time

sys.path.insert(0, "/opt/trn_rl_repo")

import numpy as np
import ml_dtypes

import concourse.bass as bass
import concourse.mybir as mybir
import concourse.tile as tile
from concourse import bacc
from concourse.masks import make_identity

F32 = mybir.dt.float32
F32R = mybir.dt.float32r
BF16 = mybir.dt.bfloat16
NPBF16 = ml_dtypes.bfloat16
P = 128
HD = 128            # head dim
NHL = 4             # query heads per core
E = 2048            # hidden
DQ = NHL * HD       # 512, local q-projection width / o-slice width
SCALE = 1.0 / np.sqrt(np.float32(HD))
REPLICA_GROUPS = [[0, 1, 2, 3], [4, 5, 6, 7]]
LAG = 4             # pv(kt-LAG) emitted after scores(kt): hides exp+mask


def r32(ap):
    return ap.bitcast(F32R)


def build_program(S=2048, reps=1, n_cores=8):
    """Build the per-core SPMD Bass program. Returns compiled nc."""
    NQC = S // 512       # 512-wide chunks along sequence
    ET = E // P          # 16 tiles along hidden

    nc = bacc.Bacc("TRN2", target_bir_lowering=False, debug=False,
                   num_devices=n_cores)

    xT_in = nc.declare_dram_parameter("xT", [E, S], BF16, isOutput=False)
    wqT_in = nc.declare_dram_parameter("wqT", [E, DQ], BF16, isOutput=False)
    wkT_in = nc.declare_dram_parameter("wkT", [E, HD], BF16, isOutput=False)
    wvT_in = nc.declare_dram_parameter("wvT", [E, HD], BF16, isOutput=False)
    woT_in = nc.declare_dram_parameter("woT", [E, DQ], BF16, isOutput=False)
    cosT_in = nc.declare_dram_parameter("cosT", [HD, S], BF16, isOutput=False)
    sinT_in = nc.declare_dram_parameter("sinT", [HD, S], BF16, isOutput=False)
    out_d = nc.declare_dram_parameter("out", [DQ, S], F32, isOutput=True)

    with tile.TileContext(nc) as tc:
        with nc.allow_low_precision(reason="bf16/f32r attention pipeline"):
            _emit(tc, nc, S, NQC, ET, reps,
                  xT_in, wqT_in, wkT_in, wvT_in, woT_in, cosT_in, sinT_in,
                  out_d)

    nc.compile()
    return nc


def _emit(tc, nc, S, NQC, ET, reps,
          xT_in, wqT_in, wkT_in, wvT_in, woT_in, cosT_in, sinT_in, out_d):
    from contextlib import ExitStack

    ctx = ExitStack()
    with ctx:
        const = ctx.enter_context(tc.tile_pool(name="const", bufs=1))
        wpool = ctx.enter_context(tc.tile_pool(name="wpool", bufs=1))
        qkv = ctx.enter_context(tc.tile_pool(name="qkv", bufs=1))
        dram = ctx.enter_context(tc.tile_pool(name="dram", bufs=1, space="DRAM"))
        xn_pool = ctx.enter_context(tc.tile_pool(name="xn", bufs=2))
        rope_pool = ctx.enter_context(tc.tile_pool(name="rope", bufs=2))
        vt_pool = ctx.enter_context(tc.tile_pool(name="vt", bufs=2))
        ex_pool = ctx.enter_context(tc.tile_pool(name="ex", bufs=6))
        sm_pool = ctx.enter_context(tc.tile_pool(name="sm", bufs=1))
        dn_pool = ctx.enter_context(tc.tile_pool(name="dn", bufs=2))
        bc_pool = ctx.enter_context(tc.tile_pool(name="bc", bufs=2))
        oh_pool = ctx.enter_context(tc.tile_pool(name="oh", bufs=3))
        af_pool = ctx.enter_context(tc.tile_pool(name="af", bufs=2))
        oc_pool = ctx.enter_context(tc.tile_pool(name="oc", bufs=1))
        pj_ps = ctx.enter_context(tc.tile_pool(name="pj_ps", bufs=2, space="PSUM"))
        sc_ps = ctx.enter_context(tc.tile_pool(name="sc_ps", bufs=3, space="PSUM"))
        pv_ps = ctx.enter_context(tc.tile_pool(name="pv_ps", bufs=1, space="PSUM"))
        dn_ps = ctx.enter_context(tc.tile_pool(name="dn_ps", bufs=1, space="PSUM"))
        oo_ps = ctx.enter_context(tc.tile_pool(name="oo_ps", bufs=1, space="PSUM"))

        # ---- constants ----
        identf = const.tile([P, P], F32)
        make_identity(nc, identf[:])
        # causal mask for the first 128 q-cols of each trimmed diagonal
        # k-tile: valid(k, q') = (q' - k) >= 0
        maskf = const.tile([P, P], F32)
        nc.gpsimd.memset(maskf[:], 1.0)
        nc.gpsimd.affine_select(
            out=maskf[:], in_=maskf[:],
            compare_op=mybir.AluOpType.is_ge,
            fill=0.0, base=0, pattern=[[1, P]],
            channel_multiplier=-1,
        )
        mask = const.tile([P, P], BF16)
        nc.vector.tensor_copy(mask[:], maskf[:])
        ones_stage = const.tile([P, P], F32)
        nc.gpsimd.memset(ones_stage[:], 1.0)
        ones_red = const.tile([P, 1], BF16)
        nc.vector.tensor_copy(ones_red[:], ones_stage[:, 0:1])
        ones_col = const.tile([1, P], F32R)
        nc.vector.tensor_copy(ones_col[:], ones_stage[0:1, :])

        # ---- persistent SBUF ----
        cosT_sb = wpool.tile([P, S], BF16)
        sinT_sb = wpool.tile([P, S], BF16)
        wqT_sb = wpool.tile([P, ET, DQ], BF16)
        wkT_sb = wpool.tile([P, ET, HD], BF16)
        wvT_sb = wpool.tile([P, ET, HD], BF16)
        woT_sb = wpool.tile([P, ET, DQ], BF16)

        QT_sb = qkv.tile([P, 2, NHL, 512], BF16)   # double-buffered per chunk
        KT_sb = qkv.tile([P, S], BF16)
        V_sb = qkv.tile([P, S // P, HD], BF16)

        # collective bounce buffers (DRAM): full chunks 0..NQC-2, plus two
        # head-pair halves for the last chunk (tail AllGather split)
        agin = [dram.tile([P, NHL * 512], BF16, name=f"agin{c}")
                for c in range(NQC - 1)]
        agout = [dram.tile([4 * P, NHL * 512], BF16, name=f"agout{c}")
                 for c in range(NQC - 1)]
        aginL = [dram.tile([P, 2 * 512], BF16, name=f"aginL{i}")
                 for i in range(2)]
        agoutL = [dram.tile([4 * P, 2 * 512], BF16, name=f"agoutL{i}")
                  for i in range(2)]

        x_r = xT_in.rearrange("(et p) s -> p et s", p=P)
        wq_r = wqT_in.rearrange("(et p) d -> p et d", p=P)
        wk_r = wkT_in.rearrange("(et p) d -> p et d", p=P)
        wv_r = wvT_in.rearrange("(et p) d -> p et d", p=P)
        wo_r = woT_in.rearrange("(et p) d -> p et d", p=P)
        out_r = out_d.rearrange("(ot p) s -> p ot s", p=P)

        def emit_oproj(g):
            sc = g % NQC
            last = (sc == NQC - 1)
            # af loaded as two head-pair halves (h 0-1, then h 2-3) so the
            # split tail AllGather can feed the first half early
            afs = []
            for half in range(2):
                afh = af_pool.tile([P, 4, 2, 512], BF16, name="afh",
                                   tag="afh")
                if last:
                    src = agoutL[half].rearrange(
                        "(r p) (h s) -> p r h s", p=P, h=2)
                else:
                    src = agout[sc].rearrange(
                        "(r p) (h s) -> p r h s", p=P,
                        h=NHL)[:, :, 2 * half:2 * half + 2, :]
                nc.sync.dma_start(afh[:], src)
                afs.append(afh)
            ms = ([m for m in range(16) if m % 4 < 2]
                  + [m for m in range(16) if m % 4 >= 2])
            outC = oc_pool.tile([P, 4, 512], F32, name="outC", tag="outC")
            for ot in range(4):
                po = oo_ps.tile([P, 512], F32, name="po", tag="po")
                for i, m in enumerate(ms):
                    r, h = m // 4, m % 4
                    nc.tensor.matmul(
                        po[:], woT_sb[:, m, ot * P:(ot + 1) * P],
                        afs[h // 2][:, r, h % 2, :],
                        start=(i == 0), stop=(i == 15))
                nc.scalar.copy(outC[:, ot, :], po[:])
            nc.sync.dma_start(out_r[:, :, sc * 512:(sc + 1) * 512], outC[:])

        xts_tiles = {}

        def ensure_x(g):
            if g in xts_tiles or g >= reps * NQC:
                return
            xt = xn_pool.tile([P, ET, 512], BF16, name="xt", tag="xt")
            sc = g % NQC
            nc.sync.dma_start(xt[:], x_r[:, :, sc * 512:(sc + 1) * 512])
            xts_tiles[g] = xt

        def load_weights():
            nc.scalar.dma_start(wqT_sb[:], wq_r[:])
            nc.scalar.dma_start(wkT_sb[:], wk_r[:])
            nc.scalar.dma_start(wvT_sb[:], wv_r[:])
            nc.scalar.dma_start(woT_sb[:], wo_r[:])
            nc.scalar.dma_start(cosT_sb[:], cosT_in[:])
            nc.scalar.dma_start(sinT_sb[:], sinT_in[:])

        load_weights()
        G = reps * NQC
        for g in range(G):
            rep, sc = divmod(g, NQC)
            s0 = sc * 512
            # ================= projection for chunk sc =================
            ensure_x(g)
            ensure_x(g + 1)   # prefetch next chunk (double-buffered pool)
            xts = xts_tiles.pop(g)

            cos_c = cosT_sb[:, s0:s0 + 512]
            sin_c = sinT_sb[:, s0:s0 + 512]
            for d6 in range(6):
                pp = pj_ps.tile([P, 512], F32, name="pp", tag="pp")
                for et in range(ET):
                    if d6 < 4:
                        lhsT = wqT_sb[:, et, d6 * HD:(d6 + 1) * HD]
                    elif d6 == 4:
                        lhsT = wkT_sb[:, et, :]
                    else:
                        lhsT = wvT_sb[:, et, :]
                    nc.tensor.matmul(pp[:], lhsT, xts[:, et, :],
                                     start=(et == 0), stop=(et == ET - 1))
                if d6 < 5:
                    dst = (QT_sb[:, g % 2, d6, :] if d6 < 4
                           else KT_sb[:, s0:s0 + 512])
                    t1 = rope_pool.tile([P, 512], BF16, name="t1", tag="t1")
                    t2 = rope_pool.tile([P, 512], BF16, name="t2", tag="t2")
                    nc.vector.tensor_tensor(t1[:], pp[:], cos_c,
                                            mybir.AluOpType.mult)
                    # sinT arrives with rows 0:64 pre-negated (host side)
                    nc.vector.tensor_tensor(t2[0:64, :], pp[64:128, :],
                                            sin_c[0:64, :],
                                            mybir.AluOpType.mult)
                    nc.vector.tensor_tensor(t2[64:128, :], pp[0:64, :],
                                            sin_c[64:128, :],
                                            mybir.AluOpType.mult)
                    nc.vector.tensor_tensor(dst, t1[:], t2[:],
                                            mybir.AluOpType.add)
                else:
                    vts = vt_pool.tile([P, 512], F32, name="vts", tag="vts")
                    nc.scalar.copy(vts[:], pp[:])
                    for st in range(4):
                        pt = pj_ps.tile([P, 512], F32, name="pt",
                                        tag="pp")[:, 0:P]
                        nc.tensor.transpose(pt[:], vts[:, st * P:(st + 1) * P],
                                            identf[:])
                        nc.scalar.copy(V_sb[:, sc * 4 + st, :], pt[:])

            # o_proj two global chunks back: the AllGather has had a full
            # chunk of compute to complete, and rep boundaries pipeline
            if g >= 2:
                emit_oproj(g - 2)

            # ================= attention for chunk sc =================
            qT = QT_sb[:, g % 2]
            for h in range(NHL):
                nkt = 4 * sc + 4
                pv = pv_ps.tile([P, 512], F32, name="pv", tag="pv")
                pden = dn_ps.tile([P, 512], F32, name="pden",
                                  tag="pden")[0:1, :]
                dacc = dn_pool.tile([P, 512], BF16, name="dacc", tag="dacc")
                exs = [None] * nkt
                qoffs = [0] * nkt

                def emit_pv(j, last):
                    nc.tensor.matmul(pv[:, qoffs[j]:512], V_sb[:, j, :],
                                     exs[j][:, 0:512 - qoffs[j]],
                                     start=(j == 0), stop=last)

                for kt in range(nkt):
                    t = kt - 4 * sc
                    qoff = 128 * t if t > 0 else 0
                    N = 512 - qoff
                    qoffs[kt] = qoff
                    ps = sc_ps.tile([P, 512], F32, name="ps", tag="ps")
                    nc.tensor.matmul(
                        ps[:, 0:N], KT_sb[:, kt * P:(kt + 1) * P],
                        qT[:, h, qoff:512], start=True, stop=True)
                    ex = ex_pool.tile([P, 512], BF16, name="ex", tag="ex")
                    exs[kt] = ex
                    nc.scalar.activation(ex[:, 0:N], ps[:, 0:N],
                                         mybir.ActivationFunctionType.Exp,
                                         scale=float(SCALE))
                    if t >= 0:
                        # staircase mask on the first 128 trimmed q-cols
                        nc.vector.tensor_tensor(
                            ex[:, 0:P], ex[:, 0:P], mask[:],
                            mybir.AluOpType.mult)
                    # denominator accumulation off the PE (DVE bf16 2x)
                    if kt == 0:
                        nc.vector.tensor_copy(dacc[:], ex[:, 0:512])
                    else:
                        nc.vector.tensor_add(dacc[:, qoff:512],
                                             dacc[:, qoff:512],
                                             ex[:, 0:N])
                    if kt >= LAG:
                        emit_pv(kt - LAG, last=False)
                for j in range(max(0, nkt - LAG), nkt):
                    emit_pv(j, last=(j == nkt - 1))

                nc.tensor.matmul(pden[:], ones_red[:], dacc[:],
                                 start=True, stop=True)
                rec = sm_pool.tile([1, 512], F32, name="rec", tag="rec")
                nc.vector.reciprocal_approx_fast(out=rec[:], in_=pden[:])
                rec_r = sm_pool.tile([1, 512], F32R, name="rec_r", tag="rec_r")
                nc.vector.tensor_copy(rec_r[:], rec[:])
                pbc = dn_ps.tile([P, 512], F32, name="pbc", tag="pden")
                nc.tensor.matmul(pbc[:], ones_col[:], rec_r[:],
                                 start=True, stop=True)
                bcr = bc_pool.tile([P, 512], BF16, name="bcr", tag="bcr")
                nc.scalar.copy(bcr[:], pbc[:])
                outH = oh_pool.tile([P, 512], BF16, name="outH", tag="outH")
                nc.vector.tensor_tensor(outH[:], pv[:], bcr[:],
                                        mybir.AluOpType.mult)
                if sc < NQC - 1:
                    nc.sync.dma_start(agin[sc][:, h * 512:(h + 1) * 512],
                                      outH[:])
                else:
                    nc.sync.dma_start(
                        aginL[h // 2][:, (h % 2) * 512:(h % 2 + 1) * 512],
                        outH[:])
                    if h % 2 == 1:
                        nc.gpsimd.collective_compute(
                            "AllGather", mybir.AluOpType.bypass,
                            replica_groups=REPLICA_GROUPS,
                            ins=[aginL[h // 2].opt()],
                            outs=[agoutL[h // 2].opt()])

            if sc == NQC - 1 and rep < reps - 1:
                load_weights()
            # ---- ship chunk sc: AllGather across the batch group ----
            if sc < NQC - 1:
                nc.gpsimd.collective_compute(
                    "AllGather", mybir.AluOpType.bypass,
                    replica_groups=REPLICA_GROUPS,
                    ins=[agin[sc].opt()],
                    outs=[agout[sc].opt()])
        if G >= 2:
            emit_oproj(G - 2)
        emit_oproj(G - 1)


# ======================= host side =======================

_CACHE = {}


def _get_program(S=2048, reps=1):
    key = (S, reps)
    if key not in _CACHE:
        _CACHE[key] = build_program(S=S, reps=reps)
    return _CACHE[key]


def make_in_maps(x, cos, sin, wq, wk, wv, wo):
    in_maps = []
    cosT = np.ascontiguousarray(cos.T.astype(NPBF16))
    sinT = sin.T.astype(np.float32).copy()
    sinT[:HD // 2, :] *= -1.0   # fold rotate_half sign into the table
    sinT = np.ascontiguousarray(sinT.astype(NPBF16))
    for c in range(8):
        b, g = c // 4, c % 4
        in_maps.append({
            "xT": np.ascontiguousarray(x[b].T.astype(NPBF16)),
            "wqT": np.ascontiguousarray(wq[g * DQ:(g + 1) * DQ, :].T.astype(NPBF16)),
            "wkT": np.ascontiguousarray(wk[g * HD:(g + 1) * HD, :].T.astype(NPBF16)),
            "wvT": np.ascontiguousarray(wv[g * HD:(g + 1) * HD, :].T.astype(NPBF16)),
            "woT": np.ascontiguousarray(wo[g * DQ:(g + 1) * DQ, :].T.astype(NPBF16)),
            "cosT": cosT,
            "sinT": sinT,
        })
    return in_maps


def assemble_output(results, B, S):
    out = np.empty((B, S, E), np.float32)
    for c in range(8):
        b, g = c // 4, c % 4
        out[b][:, g * DQ:(g + 1) * DQ] = results[c]["out"].T
    return out


# ---- inline SPMD runner (PJRT/axon), device-resident inputs ----

class SpmdRunner:
    def __init__(self, nc, n_cores):
        import jax
        from jax.sharding import Mesh, PartitionSpec
        from jax.experimental.shard_map import shard_map
        from concourse import bass2jax
        from concourse.bass2jax import _bass_exec_p, install_neuronx_cc_hook

        install_neuronx_cc_hook()
        self.jax = jax
        self.nc = nc
        self.n_cores = n_cores
        partition_name = (nc.partition_id_tensor.name
                          if nc.partition_id_tensor else None)
        in_names, out_names, out_avals = [], [], []
        zero_outs = []
        for alloc in nc.m.functions[0].allocations:
            if not isinstance(alloc, mybir.MemoryLocationSet):
                continue
            name = alloc.memorylocations[0].name
            if alloc.kind == "ExternalInput":
                if name != partition_name:
                    in_names.append(name)
            elif alloc.kind == "ExternalOutput":
                out_names.append(name)
                shape = tuple(alloc.tensor_shape)
                dtype = mybir.dt.np(alloc.dtype)
                out_avals.append(jax.core.ShapedArray(shape, dtype))
                zero_outs.append(np.zeros(shape, dtype))
        self.in_names, self.out_names = in_names, out_names
        self.out_avals, self.zero_outs = out_avals, zero_outs
        self.n_params = len(in_names)

        all_in = list(in_names) + list(out_names)
        if partition_name is not None:
            all_in.append(partition_name)

        def _body(*args):
            operands = list(args)
            if partition_name is not None:
                operands.append(bass2jax.partition_id_tensor())
            outs = _bass_exec_p.bind(
                *operands, out_avals=tuple(out_avals),
                in_names=tuple(all_in), out_names=tuple(out_names),
                lowering_input_output_aliases=(),
                sim_require_finite=True, sim_require_nnan=True, nc=nc)
            return tuple(outs)

        devices = jax.devices()[:n_cores]
        self.mesh = Mesh(np.asarray(devices), ("core",))
        n_outs = len(out_names)
        in_specs = (PartitionSpec("core"),) * (self.n_params + n_outs)
        out_specs = (PartitionSpec("core"),) * n_outs
        self.fn = jax.jit(
            shard_map(_body, mesh=self.mesh, in_specs=in_specs,
                      out_specs=out_specs, check_rep=False),
            keep_unused=True)
        self.dev_args = None

    def stage_inputs(self, in_maps):
        import jax
        from jax.sharding import PartitionSpec
        per_core = [[np.asarray(m[n]) for n in self.in_names] for m in in_maps]
        concat_in = [
            np.concatenate([per_core[c][i] for c in range(self.n_cores)], axis=0)
            for i in range(self.n_params)]
        concat_zeros = [
            np.zeros((self.n_cores * z.shape[0], *z.shape[1:]), z.dtype)
            for z in self.zero_outs]
        sharding = jax.sharding.NamedSharding(self.mesh, PartitionSpec("core"))
        self.dev_args = [jax.device_put(a, sharding)
                         for a in (*concat_in, *concat_zeros)]
        for a in self.dev_args:
            a.block_until_ready()

    def run(self):
        out_arrs = [np.asarray(o) for o in self.fn(*self.dev_args)]
        return [
            {n: out_arrs[i].reshape(self.n_cores, *self.out_avals[i].shape)[c]
             for i, n in enumerate(self.out_names)}
            for c in range(self.n_cores)]

    def time_exec(self, iters=30, warmup=3):
        import jax
        for _ in range(warmup):
            res = self.fn(*self.dev_args)
        jax.block_until_ready(res)
        t0 = time.perf_counter()
        for _ in range(iters):
            res = self.fn(*self.dev_args)
        jax.block_until_ready(res)
        t1 = time.perf_counter()
        return (t1 - t0) / iters * 1e9


_RUNNER_CACHE = {}


def get_runner(S=2048, reps=1):
    key = (S, reps)
    if key not in _RUNNER_CACHE:
        nc = _get_program(S=S, reps=reps)
        _RUNNER_CACHE[key] = SpmdRunner(nc, 8)
    return _RUNNER_CACHE[key]


def kernel(x, cos, sin, wq, wk, wv, wo):
    B, S, _ = x.shape
    runner = get_runner(S=S, reps=1)
    runner.stage_inputs(make_in_maps(x, cos, sin, wq, wk, wv, wo))
    results = runner.run()
    return assemble_output(results, B, S)


if __name__ == "__main__":
    # tiny self-test against a local numpy reference
    S = int(sys.argv[1]) if len(sys.argv) > 1 else 512
    rng = np.random.default_rng(0)
    B, H, HKV = 2, 16, 4
    x = rng.standard_normal((B, S, E), dtype=np.float32)
    cos = rng.random((S, HD), dtype=np.float32)
    sin = rng.random((S, HD), dtype=np.float32)
    sc = 0.02
    wq = (rng.standard_normal((H * HD, E), dtype=np.float32) * sc)
    wk = (rng.standard_normal((HKV * HD, E), dtype=np.float32) * sc)
    wv = (rng.standard_normal((HKV * HD, E), dtype=np.float32) * sc)
    wo = (rng.standard_normal((E, H * HD), dtype=np.float32) * sc)

    def ref(x, cos, sin, wq, wk, wv, wo):
        x64 = x.astype(np.float64)
        q = (x64 @ wq.T.astype(np.float64)).reshape(B, S, H, HD)
        k = (x64 @ wk.T.astype(np.float64)).reshape(B, S, HKV, HD)
        v = (x64 @ wv.T.astype(np.float64)).reshape(B, S, HKV, HD)

        def rot(t):
            return np.concatenate([-t[..., HD // 2:], t[..., :HD // 2]], -1)

        c = cos[:, None, :].astype(np.float64)
        s = sin[:, None, :].astype(np.float64)
        q = q * c + rot(q) * s
        k = k * c + rot(k) * s
        k = np.repeat(k, H // HKV, axis=2).transpose(0, 2, 1, 3)
        v = np.repeat(v, H // HKV, axis=2).transpose(0, 2, 1, 3)
        q = q.transpose(0, 2, 1, 3)
        scores = np.einsum("bhqd,bhkd->bhqk", q, k) / np.sqrt(HD)
        mask = np.tril(np.ones((S, S), bool))
        scores = np.where(mask, scores, -np.inf)
        scores -= scores.max(-1, keepdims=True)
        p = np.exp(scores)
        p /= p.sum(-1, keepdims=True)
        o = np.einsum("bhqk,bhkd->bhqd", p, v)
        o = o.transpose(0, 2, 1, 3).reshape(B, S, H * HD)
        return o @ wo.T.astype(np.float64)

    want = ref(x, cos, sin, wq, wk, wv, wo)
    got = kernel(x, cos, sin, wq, wk, wv, wo)
    err = np.abs(got - want).max() / np.abs(want).max()
    print(f"S={S}: rel err (absmax-relative) = {err:.3e}")


# revision 31
# speedup vs baseline: 2.3713x; 1.4920x over previous
"""Trainium2 Bass kernel for nn_Attention (dense transformer block:
QKV proj + RoPE + causal GQA attention + o_proj), SPMD over 8 NeuronCores.

Sharding: core c -> (batch b = c//4, head-group g = c%4). Each core computes
4 query heads + its kv head for one batch; per s-chunk the 4 head outputs are
AllGather'd (bf16) within the 4-core batch group and each core computes a
disjoint 512-column slice of the o_proj output for that chunk.

v2 vs v1:
- x passed host-transposed (xT [E,S]) -> no PE transposes / PSUM evacuation.
- chunk-outer pipeline: proj(sc) -> attention(sc, 4 heads) -> AllGather(sc)
  -> o_proj(sc-1), so collectives overlap the next chunk's compute.
- bf16 for QT/KT/V/probs/AllGather/o_proj operands (PE rate unchanged,
  halves collective bytes, 2x DVE mask, FWL weight loads).
- causal trim: diagonal k-tiles only compute q-columns >= 128*t.
- o_proj accumulates all 16 head-blocks in PSUM (no DVE adds / outAcc).
- reciprocal_approx_fast instead of DVE reciprocal (3.3us -> ~0.7us).
"""

import sys
import time

sys.path.insert(0, "/opt/trn_rl_repo")

import numpy as np
import ml_dtypes

import concourse.bass as bass
import concourse.mybir as mybir
import concourse.tile as tile
from concourse import bacc
from concourse.masks import make_identity

F32 = mybir.dt.float32
F32R = mybir.dt.float32r
BF16 = mybir.dt.bfloat16
NPBF16 = ml_dtypes.bfloat16
P = 128
HD = 128            # head dim
NHL = 4             # query heads per core
E = 2048            # hidden
DQ = NHL * HD       # 512, local q-projection width / o-slice width
SCALE = 1.0 / np.sqrt(np.float32(HD))
REPLICA_GROUPS = [[0, 1, 2, 3], [4, 5, 6, 7]]
LAG = 4             # pv(kt-LAG) emitted after scores(kt): hides exp+mask


def r32(ap):
    return ap.bitcast(F32R)


def build_program(S=2048, reps=1, n_cores=8):
    """Build the per-core SPMD Bass program. Returns compiled nc."""
    NQC = S // 512       # 512-wide chunks along sequence
    ET = E // P          # 16 tiles along hidden

    nc = bacc.Bacc("TRN2", target_bir_lowering=False, debug=False,
                   num_devices=n_cores)

    xT_in = nc.declare_dram_parameter("xT", [E, S], BF16, isOutput=False)
    wqT_in = nc.declare_dram_parameter("wqT", [E, DQ], BF16, isOutput=False)
    wkT_in = nc.declare_dram_parameter("wkT", [E, HD], BF16, isOutput=False)
    wvT_in = nc.declare_dram_parameter("wvT", [E, HD], BF16, isOutput=False)
    woT_in = nc.declare_dram_parameter("woT", [E, DQ], BF16, isOutput=False)
    cosT_in = nc.declare_dram_parameter("cosT", [HD, S], BF16, isOutput=False)
    sinT_in = nc.declare_dram_parameter("sinT", [HD, S], BF16, isOutput=False)
    out_d = nc.declare_dram_parameter("out", [DQ, S], F32, isOutput=True)

    with tile.TileContext(nc) as tc:
        with nc.allow_low_precision(reason="bf16/f32r attention pipeline"):
            _emit(tc, nc, S, NQC, ET, reps,
                  xT_in, wqT_in, wkT_in, wvT_in, woT_in, cosT_in, sinT_in,
                  out_d)

    nc.compile()
    return nc


def _emit(tc, nc, S, NQC, ET, reps,
          xT_in, wqT_in, wkT_in, wvT_in, woT_in, cosT_in, sinT_in, out_d):
    from contextlib import ExitStack

    ctx = ExitStack()
    with ctx:
        const = ctx.enter_context(tc.tile_pool(name="const", bufs=1))
        wpool = ctx.enter_context(tc.tile_pool(name="wpool", bufs=1))
        qkv = ctx.enter_context(tc.tile_pool(name="qkv", bufs=1))
        dram = ctx.enter_context(tc.tile_pool(name="dram", bufs=1, space="DRAM"))
        xn_pool = ctx.enter_context(tc.tile_pool(name="xn", bufs=2))
        rope_pool = ctx.enter_context(tc.tile_pool(name="rope", bufs=2))
        vt_pool = ctx.enter_context(tc.tile_pool(name="vt", bufs=2))
        ex_pool = ctx.enter_context(tc.tile_pool(name="ex", bufs=6))
        sm_pool = ctx.enter_context(tc.tile_pool(name="sm", bufs=1))
        dn_pool = ctx.enter_context(tc.tile_pool(name="dn", bufs=2))
        bc_pool = ctx.enter_context(tc.tile_pool(name="bc", bufs=2))
        oh_pool = ctx.enter_context(tc.tile_pool(name="oh", bufs=3))
        af_pool = ctx.enter_context(tc.tile_pool(name="af", bufs=2))
        oc_pool = ctx.enter_context(tc.tile_pool(name="oc", bufs=1))
        pj_ps = ctx.enter_context(tc.tile_pool(name="pj_ps", bufs=2, space="PSUM"))
        sc_ps = ctx.enter_context(tc.tile_pool(name="sc_ps", bufs=3, space="PSUM"))
        pv_ps = ctx.enter_context(tc.tile_pool(name="pv_ps", bufs=1, space="PSUM"))
        dn_ps = ctx.enter_context(tc.tile_pool(name="dn_ps", bufs=1, space="PSUM"))
        oo_ps = ctx.enter_context(tc.tile_pool(name="oo_ps", bufs=1, space="PSUM"))

        # ---- constants ----
        identf = const.tile([P, P], F32)
        make_identity(nc, identf[:])
        # causal mask for the first 128 q-cols of each trimmed diagonal
        # k-tile: valid(k, q') = (q' - k) >= 0
        maskf = const.tile([P, P], F32)
        nc.gpsimd.memset(maskf[:], 1.0)
        nc.gpsimd.affine_select(
            out=maskf[:], in_=maskf[:],
            compare_op=mybir.AluOpType.is_ge,
            fill=0.0, base=0, pattern=[[1, P]],
            channel_multiplier=-1,
        )
        mask = const.tile([P, P], BF16)
        nc.vector.tensor_copy(mask[:], maskf[:])
        ones_stage = const.tile([P, P], F32)
        nc.gpsimd.memset(ones_stage[:], 1.0)
        ones_red = const.tile([P, 1], BF16)
        nc.vector.tensor_copy(ones_red[:], ones_stage[:, 0:1])
        ones_col = const.tile([1, P], F32R)
        nc.vector.tensor_copy(ones_col[:], ones_stage[0:1, :])

        # ---- persistent SBUF ----
        cosT_sb = wpool.tile([P, S], BF16)
        sinT_sb = wpool.tile([P, S], BF16)
        wqT_sb = wpool.tile([P, ET, DQ], BF16)
        wkT_sb = wpool.tile([P, ET, HD], BF16)
        wvT_sb = wpool.tile([P, ET, HD], BF16)
        woT_sb = wpool.tile([P, ET, DQ], BF16)

        QT_sb = qkv.tile([P, 2, NHL, 512], BF16)   # double-buffered per chunk
        KT_sb = qkv.tile([P, S], BF16)
        V_sb = qkv.tile([P, S // P, HD], BF16)

        # collective bounce buffers (DRAM): full chunks 0..NQC-2, plus two
        # head-pair halves for the last chunk (tail AllGather split)
        agin = [dram.tile([P, NHL * 512], BF16, name=f"agin{c}")
                for c in range(NQC - 1)]
        agout = [dram.tile([4 * P, NHL * 512], BF16, name=f"agout{c}")
                 for c in range(NQC - 1)]
        aginL = [dram.tile([P, 2 * 512], BF16, name=f"aginL{i}")
                 for i in range(2)]
        agoutL = [dram.tile([4 * P, 2 * 512], BF16, name=f"agoutL{i}")
                  for i in range(2)]

        x_r = xT_in.rearrange("(et p) s -> p et s", p=P)
        wq_r = wqT_in.rearrange("(et p) d -> p et d", p=P)
        wk_r = wkT_in.rearrange("(et p) d -> p et d", p=P)
        wv_r = wvT_in.rearrange("(et p) d -> p et d", p=P)
        wo_r = woT_in.rearrange("(et p) d -> p et d", p=P)
        out_r = out_d.rearrange("(ot p) s -> p ot s", p=P)

        def emit_oproj(g):
            sc = g % NQC
            last = (sc == NQC - 1)
            # af loaded as two head-pair halves (h 0-1, then h 2-3) so the
            # split tail AllGather can feed the first half early
            afs = []
            for half in range(2):
                afh = af_pool.tile([P, 4, 2, 512], BF16, name="afh",
                                   tag="afh")
                if last:
                    src = agoutL[half].rearrange(
                        "(r p) (h s) -> p r h s", p=P, h=2)
                else:
                    src = agout[sc].rearrange(
                        "(r p) (h s) -> p r h s", p=P,
                        h=NHL)[:, :, 2 * half:2 * half + 2, :]
                nc.sync.dma_start(afh[:], src)
                afs.append(afh)
            ms = ([m for m in range(16) if m % 4 < 2]
                  + [m for m in range(16) if m % 4 >= 2])
            outC = oc_pool.tile([P, 4, 512], F32, name="outC", tag="outC")
            for ot in range(4):
                po = oo_ps.tile([P, 512], F32, name="po", tag="po")
                for i, m in enumerate(ms):
                    r, h = m // 4, m % 4
                    nc.tensor.matmul(
                        po[:], woT_sb[:, m, ot * P:(ot + 1) * P],
                        afs[h // 2][:, r, h % 2, :],
                        start=(i == 0), stop=(i == 15))
                nc.scalar.copy(outC[:, ot, :], po[:])
            nc.sync.dma_start(out_r[:, :, sc * 512:(sc + 1) * 512], outC[:])

        xts_tiles = {}

        def ensure_x(g):
            if g in xts_tiles or g >= reps * NQC:
                return
            xt = xn_pool.tile([P, ET, 512], BF16, name="xt", tag="xt")
            sc = g % NQC
            nc.sync.dma_start(xt[:], x_r[:, :, sc * 512:(sc + 1) * 512])
            xts_tiles[g] = xt

        def load_weights():
            nc.scalar.dma_start(wqT_sb[:], wq_r[:])
            nc.scalar.dma_start(wkT_sb[:], wk_r[:])
            nc.scalar.dma_start(wvT_sb[:], wv_r[:])
            nc.scalar.dma_start(woT_sb[:], wo_r[:])
            nc.scalar.dma_start(cosT_sb[:], cosT_in[:])
            nc.scalar.dma_start(sinT_sb[:], sinT_in[:])

        load_weights()
        G = reps * NQC
        for g in range(G):
            rep, sc = divmod(g, NQC)
            s0 = sc * 512
            # ================= projection for chunk sc =================
            ensure_x(g)
            ensure_x(g + 1)   # prefetch next chunk (double-buffered pool)
            xts = xts_tiles.pop(g)

            cos_c = cosT_sb[:, s0:s0 + 512]
            sin_c = sinT_sb[:, s0:s0 + 512]
            for d6 in range(6):
                pp = pj_ps.tile([P, 512], F32, name="pp", tag="pp")
                for et in range(ET):
                    if d6 < 4:
                        lhsT = wqT_sb[:, et, d6 * HD:(d6 + 1) * HD]
                    elif d6 == 4:
                        lhsT = wkT_sb[:, et, :]
                    else:
                        lhsT = wvT_sb[:, et, :]
                    nc.tensor.matmul(pp[:], lhsT, xts[:, et, :],
                                     start=(et == 0), stop=(et == ET - 1))
                if d6 < 5:
                    dst = (QT_sb[:, g % 2, d6, :] if d6 < 4
                           else KT_sb[:, s0:s0 + 512])
                    t1 = rope_pool.tile([P, 512], BF16, name="t1", tag="t1")
                    t2 = rope_pool.tile([P, 512], BF16, name="t2", tag="t2")
                    nc.vector.tensor_tensor(t1[:], pp[:], cos_c,
                                            mybir.AluOpType.mult)
                    # sinT arrives with rows 0:64 pre-negated (host side)
                    nc.vector.tensor_tensor(t2[0:64, :], pp[64:128, :],
                                            sin_c[0:64, :],
                                            mybir.AluOpType.mult)
                    nc.vector.tensor_tensor(t2[64:128, :], pp[0:64, :],
                                            sin_c[64:128, :],
                                            mybir.AluOpType.mult)
                    nc.vector.tensor_tensor(dst, t1[:], t2[:],
                                            mybir.AluOpType.add)
                else:
                    vts = vt_pool.tile([P, 512], F32, name="vts", tag="vts")
                    nc.scalar.copy(vts[:], pp[:])
                    for st in range(4):
                        pt = pj_ps.tile([P, 512], F32, name="pt",
                                        tag="pp")[:, 0:P]
                        nc.tensor.transpose(pt[:], vts[:, st * P:(st + 1) * P],
                                            identf[:])
                        nc.scalar.copy(V_sb[:, sc * 4 + st, :], pt[:])

            # o_proj two global chunks back: the AllGather has had a full
            # chunk of compute to complete, and rep boundaries pipeline
            if g >= 2:
                emit_oproj(g - 2)

            # ================= attention for chunk sc =================
            qT = QT_sb[:, g % 2]
            for h in range(NHL):
                nkt = 4 * sc + 4
                pv = pv_ps.tile([P, 512], F32, name="pv", tag="pv")
                pden = dn_ps.tile([P, 512], F32, name="pden",
                                  tag="pden")[0:1, :]
                dacc = dn_pool.tile([P, 512], BF16, name="dacc", tag="dacc")
                exs = [None] * nkt
                qoffs = [0] * nkt

                def emit_pv(j, last):
                    nc.tensor.matmul(pv[:, qoffs[j]:512], V_sb[:, j, :],
                                     exs[j][:, 0:512 - qoffs[j]],
                                     start=(j == 0), stop=last)

                for kt in range(nkt):
                    t = kt - 4 * sc
                    qoff = 128 * t if t > 0 else 0
                    N = 512 - qoff
                    qoffs[kt] = qoff
                    ps = sc_ps.tile([P, 512], F32, name="ps", tag="ps")
                    nc.tensor.matmul(
                        ps[:, 0:N], KT_sb[:, kt * P:(kt + 1) * P],
                        qT[:, h, qoff:512], start=True, stop=True)
                    ex = ex_pool.tile([P, 512], BF16, name="ex", tag="ex")
                    exs[kt] = ex
                    nc.scalar.activation(ex[:, 0:N], ps[:, 0:N],
                                         mybir.ActivationFunctionType.Exp,
                                         scale=float(SCALE))
                    if t >= 0:
                        # staircase mask on the first 128 trimmed q-cols
                        nc.vector.tensor_tensor(
                            ex[:, 0:P], ex[:, 0:P], mask[:],
                            mybir.AluOpType.mult)
                    # denominator accumulation off the PE (DVE bf16 2x)
                    if kt == 0:
                        nc.vector.tensor_copy(dacc[:], ex[:, 0:512])
                    else:
                        nc.vector.tensor_add(dacc[:, qoff:512],
                                             dacc[:, qoff:512],
                                             ex[:, 0:N])
                    if kt >= LAG:
                        emit_pv(kt - LAG, last=False)
                for j in range(max(0, nkt - LAG), nkt):
                    emit_pv(j, last=(j == nkt - 1))

                nc.tensor.matmul(pden[:], ones_red[:], dacc[:],
                                 start=True, stop=True)
                rec = sm_pool.tile([1, 512], F32, name="rec", tag="rec")
                nc.vector.reciprocal_approx_fast(out=rec[:], in_=pden[:])
                rec_r = sm_pool.tile([1, 512], F32R, name="rec_r", tag="rec_r")
                nc.vector.tensor_copy(rec_r[:], rec[:])
                pbc = dn_ps.tile([P, 512], F32, name="pbc", tag="pden")
                nc.tensor.matmul(pbc[:], ones_col[:], rec_r[:],
                                 start=True, stop=True)
                bcr = bc_pool.tile([P, 512], BF16, name="bcr", tag="bcr")
                nc.scalar.copy(bcr[:], pbc[:])
                outH = oh_pool.tile([P, 512], BF16, name="outH", tag="outH")
                nc.vector.tensor_tensor(outH[:], pv[:], bcr[:],
                                        mybir.AluOpType.mult)
                if sc < NQC - 1:
                    nc.sync.dma_start(agin[sc][:, h * 512:(h + 1) * 512],
                                      outH[:])
                else:
                    nc.sync.dma_start(
                        aginL[h // 2][:, (h % 2) * 512:(h % 2 + 1) * 512],
                        outH[:])
                    if h % 2 == 1:
                        nc.gpsimd.collective_compute(
                            "AllGather", mybir.AluOpType.bypass,
                            replica_groups=REPLICA_GROUPS,
                            ins=[aginL[h // 2].opt()],
                            outs=[agoutL[h // 2].opt()])

            if sc == NQC - 1 and rep < reps - 1:
                load_weights()
            # ---- ship chunk sc: AllGather across the batch group ----
            if sc < NQC - 1:
                nc.gpsimd.collective_compute(
                    "AllGather", mybir.AluOpType.bypass,
                    replica_groups=REPLICA_GROUPS,
                    ins=[agin[sc].opt()],
                    outs=[agout[sc].opt()])
        if G >= 2:
            emit_oproj(G - 2)
        emit_oproj(G - 1)


# ======================= host side =======================

_CACHE = {}


def _get_program(S=2048, reps=1):
    key = (S, reps)
    if key not in _CACHE:
        _CACHE[key] = build_program(S=S, reps=reps)
    return _CACHE[key]


def make_in_maps(x, cos, sin, wq, wk, wv, wo):
    in_maps = []
    cosT = np.ascontiguousarray(cos.T.astype(NPBF16))
    sinT = sin.T.astype(np.float32).copy()
    sinT[:HD // 2, :] *= -1.0   # fold rotate_half sign into the table
    sinT = np.ascontiguousarray(sinT.astype(NPBF16))
    for c in range(8):
        b, g = c // 4, c % 4
        in_maps.append({
            "xT": np.ascontiguousarray(x[b].T.astype(NPBF16)),
            "wqT": np.ascontiguousarray(wq[g * DQ:(g + 1) * DQ, :].T.astype(NPBF16)),
            "wkT": np.ascontiguousarray(wk[g * HD:(g + 1) * HD, :].T.astype(NPBF16)),
            "wvT": np.ascontiguousarray(wv[g * HD:(g + 1) * HD, :].T.astype(NPBF16)),
            "woT": np.ascontiguousarray(wo[g * DQ:(g + 1) * DQ, :].T.astype(NPBF16)),
            "cosT": cosT,
            "sinT": sinT,
        })
    return in_maps


def assemble_output(results, B, S):
    out = np.empty((B, S, E), np.float32)
    for c in range(8):
        b, g = c // 4, c % 4
        out[b][:, g * DQ:(g + 1) * DQ] = results[c]["out"].T
    return out


# ---- inline SPMD runner (PJRT/axon), device-resident inputs ----

class SpmdRunner:
    def __init__(self, nc, n_cores):
        import jax
        from jax.sharding import Mesh, PartitionSpec
        from jax.experimental.shard_map import shard_map
        from concourse import bass2jax
        from concourse.bass2jax import _bass_exec_p, install_neuronx_cc_hook

        install_neuronx_cc_hook()
        self.jax = jax
        self.nc = nc
        self.n_cores = n_cores
        partition_name = (nc.partition_id_tensor.name
                          if nc.partition_id_tensor else None)
        in_names, out_names, out_avals = [], [], []
        zero_outs = []
        for alloc in nc.m.functions[0].allocations:
            if not isinstance(alloc, mybir.MemoryLocationSet):
                continue
            name = alloc.memorylocations[0].name
            if alloc.kind == "ExternalInput":
                if name != partition_name:
                    in_names.append(name)
            elif alloc.kind == "ExternalOutput":
                out_names.append(name)
                shape = tuple(alloc.tensor_shape)
                dtype = mybir.dt.np(alloc.dtype)
                out_avals.append(jax.core.ShapedArray(shape, dtype))
                zero_outs.append(np.zeros(shape, dtype))
        self.in_names, self.out_names = in_names, out_names
        self.out_avals, self.zero_outs = out_avals, zero_outs
        self.n_params = len(in_names)

        all_in = list(in_names) + list(out_names)
        if partition_name is not None:
            all_in.append(partition_name)

        def _body(*args):
            operands = list(args)
            if partition_name is not None:
                operands.append(bass2jax.partition_id_tensor())
            outs = _bass_exec_p.bind(
                *operands, out_avals=tuple(out_avals),
                in_names=tuple(all_in), out_names=tuple(out_names),
                lowering_input_output_aliases=(),
                sim_require_finite=True, sim_require_nnan=True, nc=nc)
            return tuple(outs)

        devices = jax.devices()[:n_cores]
        self.mesh = Mesh(np.asarray(devices), ("core",))
        n_outs = len(out_names)
        in_specs = (PartitionSpec("core"),) * (self.n_params + n_outs)
        out_specs = (PartitionSpec("core"),) * n_outs
        self.fn = jax.jit(
            shard_map(_body, mesh=self.mesh, in_specs=in_specs,
                      out_specs=out_specs, check_rep=False),
            keep_unused=True)
        self.dev_args = None

    def stage_inputs(self, in_maps):
        import jax
        from jax.sharding import PartitionSpec
        per_core = [[np.asarray(m[n]) for n in self.in_names] for m in in_maps]
        concat_in = [
            np.concatenate([per_core[c][i] for c in range(self.n_cores)], axis=0)
            for i in range(self.n_params)]
        concat_zeros = [
            np.zeros((self.n_cores * z.shape[0], *z.shape[1:]), z.dtype)
            for z in self.zero_outs]
        sharding = jax.sharding.NamedSharding(self.mesh, PartitionSpec("core"))
        self.dev_args = [jax.device_put(a, sharding)
                         for a in (*concat_in, *concat_zeros)]
        for a in self.dev_args:
            a.block_until_ready()

    def run(self):
        out_arrs = [np.asarray(o) for o in self.fn(*self.dev_args)]
        return [
            {n: out_arrs[i].reshape(self.n_cores, *self.out_avals[i].shape)[c]
             for i, n in enumerate(self.out_names)}
            for c in range(self.n_cores)]

    def time_exec(self, iters=30, warmup=3):
        import jax
        for _ in range(warmup):
            res = self.fn(*self.dev_args)
        jax.block_until_ready(res)
        t0 = time.perf_counter()
        for _ in range(iters):
            res = self.fn(*self.dev_args)
        jax.block_until_ready(res)
        t1 = time.perf_counter()
        return (t1 - t0) / iters * 1e9


_RUNNER_CACHE = {}


def get_runner(S=2048, reps=1):
    key = (S, reps)
    if key not in _RUNNER_CACHE:
        nc = _get_program(S=S, reps=reps)
        _RUNNER_CACHE[key] = SpmdRunner(nc, 8)
    return _RUNNER_CACHE[key]


def kernel(x, cos, sin, wq, wk, wv, wo):
    B, S, _ = x.shape
    runner = get_runner(S=S, reps=1)
    runner.stage_inputs(make_in_maps(x, cos, sin, wq, wk, wv, wo))
    results = runner.run()
    return assemble_output(results, B, S)


if __name__ == "__main__":
    # tiny self-test against a local numpy reference
    S = int(sys.argv[1]) if len(sys.argv) > 1 else 512
    rng = np.random.default_rng(0)
    B, H, HKV = 2, 16, 4
    x = rng.standard_normal((B, S, E), dtype=np.float32)
    cos = rng.random((S, HD), dtype=np.float32)
    sin = rng.random((S, HD), dtype=np.float32)
    sc = 0.02
    wq = (rng.standard_normal((H * HD, E), dtype=np.float32) * sc)
    wk = (rng.standard_normal((HKV * HD, E), dtype=np.float32) * sc)
    wv = (rng.standard_normal((HKV * HD, E), dtype=np.float32) * sc)
    wo = (rng.standard_normal((E, H * HD), dtype=np.float32) * sc)

    def ref(x, cos, sin, wq, wk, wv, wo):
        x64 = x.astype(np.float64)
        q = (x64 @ wq.T.astype(np.float64)).reshape(B, S, H, HD)
        k = (x64 @ wk.T.astype(np.float64)).reshape(B, S, HKV, HD)
        v = (x64 @ wv.T.astype(np.float64)).reshape(B, S, HKV, HD)

        def rot(t):
            return np.concatenate([-t[..., HD // 2:], t[..., :HD // 2]], -1)

        c = cos[:, None, :].astype(np.float64)
        s = sin[:, None, :].astype(np.float64)
        q = q * c + rot(q) * s
        k = k * c + rot(k) * s
        k = np.repeat(k, H // HKV, axis=2).transpose(0, 2, 1, 3)
        v = np.repeat(v, H // HKV, axis=2).transpose(0, 2, 1, 3)
        q = q.transpose(0, 2, 1, 3)
        scores = np.einsum("bhqd,bhkd->bhqk", q, k) / np.sqrt(HD)
        mask = np.tril(np.ones((S, S), bool))
        scores = np.where(mask, scores, -np.inf)
        scores -= scores.max(-1, keepdims=True)
        p = np.exp(scores)
        p /= p.sum(-1, keepdims=True)
        o = np.einsum("bhqk,bhkd->bhqd", p, v)
        o = o.transpose(0, 2, 1, 3).reshape(B, S, H * HD)
        return o @ wo.T.astype(np.float64)

    want = ref(x, cos, sin, wq, wk, wv, wo)
    got = kernel(x, cos, sin, wq, wk, wv, wo)
    err = np.abs(got - want).max() / np.abs(want).max()
    print(f"S={S}: rel err (absmax-relative) = {err:.3e}")
